# revision 64
# baseline (speedup 1.0000x reference)
"""CausalWanS2V self-attention — 8-core head-sharded Trainium2 Bass kernel.

Layout strategy (per core c, heads 2c..2c+1), v2 (bf16 + paired-exp):
  - All matmul operands bf16 (PSUM accumulation stays f32), so every matmul
    runs at 1 cycle/row regardless of chunk width and DMA bytes are halved.
  - q/k/v projections head-dim-major as before: qT/kT [hd=128, s=720] from
    host-transposed W^T / x^T tiles.
  - qk RMSNorm over the full 2048-dim vector via AllGather of per-core
    square-sum rows [1, 720] (bf16), summed with a Pool partition_all_reduce,
    rsqrt = exp(-0.5*ln(ms)) on ACT (stays in the natural_log_exp table set),
    then a Pool partition_broadcast feeds the per-column 1/rms factors to DVE.
  - Attention in S^T layout with PAIRED k-tiles: two 128-row chunks of cache
    positions share one PSUM tile [128, 1440] and one exp instruction,
    amortizing the ACT per-instruction overhead (the sweep is exp-throughput
    bound).  S chunks split at PSUM bank boundaries: tile A (0:512, 512:720),
    tile B (720:1024, 1024:1440).
  - Denominators: E pairs accumulate into a bf16 acc [128, 1440] on DVE
    (2-byte 2x mode); final column sums via Pool partition_all_reduce, then
    reciprocal + Pool partition_broadcast, all off the PE/ACT critical path.
  - Projections and RoPE all run before the first sweep; the k-norm collective
    round-trip and rk final multiplies are emitted as guarded fillers inside
    head 0's sweep (their deadline is the new-token tiles at the sweep end).
  - o-projection per-core partial [720, 2048]; host sums the 8 partials + o_b.
"""
import sys

sys.path.insert(0, "/opt/trn_rl_repo")

from collections import defaultdict

import numpy as np
import concourse.bass as bass
import concourse.bass_isa as bass_isa
import concourse.mybir as mybir
import concourse.tile as tile
from concourse import bacc
from concourse import bass_utils

f32 = mybir.dt.float32
bf16 = mybir.dt.bfloat16
f8 = mybir.dt.float8e4
DR = mybir.MatmulPerfMode.DoubleRowSwInterleave
AF = mybir.ActivationFunctionType
RADD = bass_isa.ReduceOp.add

# problem constants (hardcoded per contract)
SEQ = 720
DIM = 2048
NH = 16
HD = 128
CACHE = 11520
N_CORES = 8
HPC = NH // N_CORES        # heads per core = 2
HDC = HPC * HD             # 256 out dims per core
OLD = CACHE - SEQ          # 10800 old cache rows
SLAB = 2160                # kpos per DMA slab (5 slabs of 16*128+112)
NSLAB = OLD // SLAB
VT = (SLAB + 127) // 128   # 17 v-tiles per slab
SM_SCALE = float(HD) ** -0.5
EPS = 1e-6
KT = DIM // 128            # 16 contraction tiles
GP = KT // 2               # 8 DoubleRow contraction pairs for the q/k proj
W8S = 64.0                 # host pre-scale of fp8 q/k weights (rmsnorm-invariant)
PW = 1536                  # PSUM pair tile width (3 banks); cols 0:1440 used

DEBUG = False

# s-tiles of 720: 5 full 128s + one 80
S_TILES = [(i * 128, min(128, SEQ - i * 128)) for i in range((SEQ + 127) // 128)]
# in-bank chunking of a 720-col range starting at psum col 0 / col 720
CHUNKS_A = ((0, 512), (512, 208))
CHUNKS_B = ((720, 304), (1024, 416))
# chunking for [*, 720] psum tiles (projections etc.)
CHUNKS_720 = ((0, 512), (512, 208))


def _emit(nc, tc, d):
    """Emit the per-core program. d = dict of dram tensor handles."""
    ap = {k: v.ap() for k, v in d.items()}

    with tc.tile_pool(name="p0", bufs=1) as p0, \
         tc.tile_pool(name="dram", bufs=1, space="DRAM") as dpool, \
         tc.tile_pool(name="pa", bufs=1) as pa, \
         tc.tile_pool(name="att", bufs=2) as att, \
         tc.tile_pool(name="epool", bufs=13) as epool, \
         tc.tile_pool(name="osb", bufs=3) as osb, \
         tc.tile_pool(name="psS", bufs=2, space="PSUM") as psS, \
         tc.tile_pool(name="psO", bufs=1, space="PSUM") as psO:

        # ---- persistent tiles ----
        rq = [p0.tile([128, SEQ], bf16, tag=f"rq{h}", name=f"rq{h}") for h in range(HPC)]
        rk = [p0.tile([128, SEQ], bf16, tag=f"rk{h}", name=f"rk{h}") for h in range(HPC)]
        vs = [p0.tile([128, HDC], bf16, tag=f"vs{st}", name=f"vs{st}") for st in range(len(S_TILES))]
        OT = [p0.tile([128, SEQ], bf16, tag=f"ot{h}", name=f"ot{h}") for h in range(HPC)]
        accs = [p0.tile([128, 1440], bf16, tag=f"acc{h}", name=f"acc{h}")
                for h in range(HPC)]
        ones_col = p0.tile([128, 1], bf16, tag="ones_col")
        ones_row = p0.tile([1, 128], bf16, tag="ones_row")
        ones_row_f = p0.tile([1, 128], f32, tag="ones_row_f")
        one_one = p0.tile([1, 1], f32, tag="one_one")
        rec_col = [p0.tile([128, 8], f32, tag=f"rec{h}", name=f"rec{h}")
                   for h in range(HPC)]
        eps_t = p0.tile([1, 1], f32, tag="eps_t")
        prewarm = p0.tile([1, 1], f32, tag="prewarm")
        qb = {(tn, h): p0.tile([128, SEQ], bf16, tag=f"qb{tn}{h}", name=f"qb{tn}{h}")
              for tn in ("q", "k") for h in range(HPC)}
        gth = {tn: p0.tile([N_CORES, SEQ], bf16, tag=f"gth{tn}", name=f"gth{tn}")
               for tn in ("q", "k")}
        gsum = {tn: p0.tile([N_CORES, SEQ], f32, tag=f"gsum{tn}",
                            name=f"gsum{tn}") for tn in ("q", "k")}
        ln_t = {tn: p0.tile([1, SEQ], f32, tag=f"ln{tn}", name=f"ln{tn}")
                for tn in ("q", "k")}
        fbt = {tn: p0.tile([128, SEQ], bf16, tag=f"fbt{tn}", name=f"fbt{tn}")
               for tn in ("q", "k")}
        recipf = {tn: p0.tile([1, SEQ], bf16, tag=f"rf{tn}", name=f"rf{tn}")
                  for tn in ("q", "k")}
        owt = p0.tile([128, HPC, DIM], bf16, tag="owt")

        nc.gpsimd.memset(eps_t[:], EPS)
        nc.gpsimd.memset(ones_col[:], 1.0)
        nc.gpsimd.memset(ones_row[:], 1.0)
        nc.gpsimd.memset(ones_row_f[:], 1.0)
        nc.gpsimd.memset(one_one[:], 1.0)
        # pre-load the natural_log_exp table set while DMAs stream
        nc.scalar.activation(prewarm[:], eps_t[:], AF.Exp)

        # ---- phase A loads ----
        wq = pa.tile([128, KT, HDC], bf16, tag="wq")
        wk = pa.tile([128, KT, HDC], bf16, tag="wk")
        wv = pa.tile([128, KT, HDC], bf16, tag="wv")
        xt = pa.tile([128, KT, SEQ], bf16, tag="xt")
        cw = {nm: pa.tile([128, HPC * SEQ], bf16, tag=nm, name=nm)
              for nm in ("cosq", "sinq", "cosk", "sink")}
        bias_t = pa.tile([128, 4], f32, tag="bias")
        vb_t = pa.tile([1, HDC], bf16, tag="vb")
        ones_vr = pa.tile([1, SEQ], bf16, tag="ones_vr")

        x_r = ap["xT"].rearrange("(g p) s -> p g s", p=128)
        w_rs = {n: ap[n].rearrange("(g p) n -> p g n", p=128)
                for n in ("wqT", "wkT", "wvT")}
        # DMA issue order == SP-FIFO service order (single HWDGE device in
        # the cost model): q-proj inputs fine-grained, then k weights, first
        # half of slab-0 k (the sweep-start gate), q-rope tables, rest of
        # slab 0, v weights, k-rope tables.  owt is issued mid-sweep-0.
        # coarse chunks amortize the serialized per-DMA DGE overhead; each
        # wq chunk lands just before the xt chunks it multiplies
        nc.sync.dma_start(wq[:, 0:8, :], w_rs["wqT"][:, 0:8, :])
        nc.sync.dma_start(xt[:, 0:4, :], x_r[:, 0:4, :])
        nc.sync.dma_start(bias_t[:], ap["qk_bias"])
        nc.sync.dma_start(xt[:, 4:8, :], x_r[:, 4:8, :])
        nc.sync.dma_start(wq[:, 8:16, :], w_rs["wqT"][:, 8:16, :])
        nc.sync.dma_start(xt[:, 8:12, :], x_r[:, 8:12, :])
        nc.sync.dma_start(xt[:, 12:16, :], x_r[:, 12:16, :])
        nc.sync.dma_start(cw["cosq"][:], ap["cosq"])
        nc.sync.dma_start(cw["sinq"][:], ap["sinq"])
        nc.sync.dma_start(wk[:, 0:8, :], w_rs["wkT"][:, 0:8, :])
        nc.sync.dma_start(wk[:, 8:16, :], w_rs["wkT"][:, 8:16, :])
        # slab 0 of head 0 preloaded into the att rotation (sweep-start gate);
        # the first half is the gate, the rest rides the ACT queue later.
        ks0 = att.tile([128, SLAB], bf16, tag="ks", name="ks00")
        vsl0 = att.tile([128, VT, HD], bf16, tag="vsl", name="vsl00")
        nc.sync.dma_start(ks0[:, 0:1024], ap["kTold"][0, :, 0:1024])
        nc.sync.dma_start(vb_t[:], ap["v_bias"])
        nc.sync.dma_start(ones_vr[:], ap["ones_vr"])
        nc.sync.dma_start(cw["cosk"][:], ap["cosk"])
        nc.sync.dma_start(cw["sink"][:], ap["sink"])

        def late_loads():
            nc.sync.dma_start(owt[:], ap["owT"].rearrange("(h p) n -> p h n", p=128))

        # ---- q then k projections, square-sums, collectives launched ASAP ----
        def proj_qk(tn, wt, ti):
            # both heads' psums allocated upfront so neither waits on the
            # other's eviction; the row reduction reuses head 0's slot
            pss = [psS.tile([128, PW], f32, tag="pair", name=f"ps_{tn}{h}")
                   for h in range(HPC)]
            for g in range(KT):
                for h in range(HPC):
                    for off, n in CHUNKS_720:
                        nc.tensor.matmul(
                            pss[h][:, off:off + n],
                            wt[:, g, h * HD:(h + 1) * HD],
                            xt[:, g, off:off + n],
                            start=(g == 0), stop=(g == KT - 1))
            sqs = []
            for h in range(HPC):
                nc.vector.tensor_scalar_add(qb[(tn, h)][:], pss[h][:, 0:SEQ],
                                            bias_t[:, 2 * ti + h:2 * ti + h + 1])
                sq = pa.tile([128, SEQ], bf16, tag=f"sq{h}", name=f"sq{tn}{h}")
                sqs.append(sq)
                nc.vector.tensor_mul(sq[:], qb[(tn, h)][:], qb[(tn, h)][:])
            row_ps = psO.tile([128, SEQ], f32, tag="o", name=f"row_{tn}")
            for h in range(HPC):
                for off, n in CHUNKS_720:
                    nc.tensor.matmul(row_ps[0:1, off:off + n], ones_col[:],
                                     sqs[h][:, off:off + n],
                                     start=(h == 0), stop=(h == HPC - 1))
            partial_sb = pa.tile([1, SEQ], bf16, tag=f"partial{tn}",
                                 name=f"partial{tn}")
            nc.vector.tensor_copy(partial_sb[0:1, :], row_ps[0:1, 0:SEQ])
            partials[tn] = partial_sb

        partials = {}

        def launch_collective(tn):
            bounce_in = dpool.tile([1, SEQ], bf16, name=f"bin{tn}")
            bounce_out = dpool.tile([N_CORES, SEQ], bf16, name=f"bout{tn}")
            # ACT-queue DMAs bypass the loaded SP FIFO (pre-sweep ACT is
            # idle); the k-side return must NOT ride ACT (it would
            # head-block the sweep exps until the collective lands).
            (nc.scalar if tn == "q" else nc.sync).dma_start(
                bounce_in[:], partials[tn][:])
            nc.gpsimd.collective_compute(
                "AllGather", mybir.AluOpType.bypass,
                replica_groups=[list(range(N_CORES))],
                ins=[bounce_in.opt()], outs=[bounce_out.opt()])
            if tn == "q":
                nc.scalar.dma_start(gth[tn][:], bounce_out[:])
            else:
                nc.sync.dma_start(gth[tn][:], bounce_out[:])

        proj_qk("q", wq, 0)

        # ---- RoPE: qb <- qb*cosW + swap(qb)*sinW; the pairwise partition
        # swap runs as two stride-2 sbuf->sbuf DMAs (no PE, no PSUM), so the
        # k-rope can run as a sweep filler.
        qbsw_t = {(tn, h): pa.tile([128, SEQ], bf16, tag=f"qbsw{tn}{h}",
                                   name=f"qbsw{tn}{h}")
                  for tn in ("q", "k") for h in range(HPC)}

        def rope_swap(tn, h):
            # q pre-sweep on the idle ACT queue (bypasses the SP FIFO); k is
            # not latency-critical and must keep off ACT (exp head-blocking).
            eng = nc.scalar if tn == "q" else nc.sync
            src, dst = qb[(tn, h)], qbsw_t[(tn, h)]
            eng.dma_start(dst[0:127:2, :], src[1:128:2, :])
            eng.dma_start(dst[1:128:2, :], src[0:127:2, :])

        def rope_muls(tn, h):
            cos_t = cw["cosq" if tn == "q" else "cosk"]
            sin_t = cw["sinq" if tn == "q" else "sink"]
            qbsw = qbsw_t[(tn, h)]
            t1 = pa.tile([128, SEQ], bf16, tag=f"t1{tn}{h}", name=f"t1{tn}{h}")
            nc.vector.tensor_mul(t1[:], qb[(tn, h)][:],
                                 cos_t[:, h * SEQ:(h + 1) * SEQ])
            nc.vector.tensor_mul(qbsw[:], qbsw[:],
                                 sin_t[:, h * SEQ:(h + 1) * SEQ])
            nc.vector.tensor_add(qb[(tn, h)][:], t1[:], qbsw[:])

        def rope(tn, h):
            rope_swap(tn, h)
            rope_muls(tn, h)

        # ---- norm factors: rsqrt(mean sq + eps) via Pool reduce + ACT ----
        def norm_factors(tn, pool=None):
            nc.gpsimd.partition_all_reduce(gsum[tn][:], gth[tn][:],
                                           channels=N_CORES, reduce_op=RADD)
            nc.scalar.activation(ln_t[tn][:], gsum[tn][0:1, :], AF.Ln,
                                 scale=1.0 / DIM, bias=eps_t[:])
            nc.scalar.activation(recipf[tn][:], ln_t[tn][:], AF.Exp,
                                 scale=-0.5)
            nc.gpsimd.partition_broadcast(fbt[tn][:], recipf[tn][0:1, :])

        def final_mul(tn):
            out_t = rq if tn == "q" else rk
            for h in range(HPC):
                nc.vector.tensor_mul(out_t[h][:], qb[(tn, h)][:], fbt[tn][:])

        # rope(q) with ACT evictions (ACT idle pre-sweep), then the norm-q
        # chain and rq finalization; the k-projection, v-projection and
        # k-RoPE all fill the q-collective round trip.
        rope("q", 0)
        rope("q", 1)
        launch_collective("q")
        # bulk slab-0 tail rides the ACT queue behind the q-norm bounce
        nc.scalar.dma_start(ks0[:, 1024:SLAB], ap["kTold"][0, :, 1024:SLAB])
        nc.scalar.dma_start(vsl0[:], ap["vold"][0, 0])
        proj_qk("k", wk, 1)

        # ---- v projection: one head x one s-tile per filler; accumulates in
        # the spare bank-1 region (cols 768:896) of the live o_ps tile, which
        # is only safe before the sweep's first PV (whose start=True zeroes
        # banks 0-1 of the slot) -- fillers go in the first pairs.
        def v_tile_h(st, h):
            s0, m = S_TILES[st]
            vp = state["o_ps"]
            for g in range(KT):
                nc.tensor.matmul(vp[0:m, 768:896], xt[:, g, s0:s0 + m],
                                 wv[:, g, h * HD:(h + 1) * HD],
                                 start=(g == 0), stop=False)
            nc.tensor.matmul(vp[0:m, 768:896], ones_vr[0:1, s0:s0 + m],
                             vb_t[:, h * HD:(h + 1) * HD], start=False, stop=True)
            nc.vector.tensor_copy(vs[st][0:m, h * HD:(h + 1) * HD],
                                  vp[0:m, 768:896])

        # ================= attention sweeps =================
        state = {"o_ps": None, "first": True}

        def emit_S_exp(pair):
            (kA, vA, mA) = pair[0]
            s_ps = psS.tile([128, PW], f32, tag="pair")
            for off, n in CHUNKS_A:
                nc.tensor.matmul(s_ps[0:mA, off:off + n], kA,
                                 rq[state["h"]][:, off:off + n],
                                 start=True, stop=True)
            if len(pair) > 1:
                (kB, vB, mB) = pair[1]
                for off, n in CHUNKS_B:
                    nc.tensor.matmul(s_ps[0:mB, off:off + n], kB,
                                     rq[state["h"]][:, off - 720:off - 720 + n],
                                     start=True, stop=True)
            else:
                mB = None
            e_t = epool.tile([128, PW], bf16, tag="e")
            if mB is None:
                nc.scalar.activation(e_t[0:mA, 0:SEQ], s_ps[0:mA, 0:SEQ],
                                     AF.Exp, scale=SM_SCALE)
            else:
                mm = max(mA, mB)
                nc.scalar.activation(e_t[0:mm, 0:1440], s_ps[0:mm, 0:1440],
                                     AF.Exp, scale=SM_SCALE)
            return e_t

        def emit_PV_acc(pair, e_t, last):
            o_ps = state["o_ps"]
            (kA, vA, mA) = pair[0]
            mB = pair[1][2] if len(pair) > 1 else None
            for off, n in CHUNKS_720:
                nc.tensor.matmul(o_ps[:, off:off + n], vA,
                                 e_t[0:mA, off:off + n],
                                 start=state["first"], stop=(last and mB is None))
            if mB is not None:
                vB = pair[1][1]
                for off, n in CHUNKS_720:
                    nc.tensor.matmul(o_ps[:, off:off + n], vB,
                                     e_t[0:mB, 720 + off:720 + off + n],
                                     start=False, stop=last)
            state["first"] = False
            # denominator accumulation on DVE (bf16 2x)
            acc = accs[state["h"]]
            eng = nc.vector
            if state["acc_first"]:
                state["acc_first"] = False
                if mB is not None and mA == mB:
                    eng.tensor_copy(acc[0:mA, :], e_t[0:mA, 0:1440])
                else:
                    eng.tensor_copy(acc[0:mA, 0:SEQ], e_t[0:mA, 0:SEQ])
                    if mB is not None:
                        eng.tensor_copy(acc[0:mB, 720:1440], e_t[0:mB, 720:1440])
            else:
                if mB is not None and mA == mB:
                    eng.tensor_add(acc[0:mA, :], acc[0:mA, :], e_t[0:mA, 0:1440])
                else:
                    eng.tensor_add(acc[0:mA, 0:SEQ], acc[0:mA, 0:SEQ],
                                   e_t[0:mA, 0:SEQ])
                    if mB is not None:
                        eng.tensor_add(acc[0:mB, 720:1440], acc[0:mB, 720:1440],
                                       e_t[0:mB, 720:1440])

        # ---- denominators, no PSUM: evict the unnormalized O^T first (frees
        # the psO slot), then reduce acc on the idle Pool engine, reciprocal
        # + partition-broadcast, and scale OT in place on DVE.
        dred = p0.tile([128, 1440], f32, tag="dred")
        dsum = p0.tile([1, SEQ], f32, tag="dsum")

        def evict_OTu(h, o_ps):
            nc.vector.tensor_copy(OT[h][:], o_ps[:, 0:SEQ])

        def den_chain(h):
            nc.gpsimd.partition_all_reduce(dred[:], accs[h][:],
                                           channels=128, reduce_op=RADD)
            nc.vector.tensor_add(dsum[:], dred[0:1, 0:SEQ],
                                 dred[0:1, 720:1440])
            recd = att.tile([1, SEQ], bf16, tag="rec_d", name=f"rec_d{h}")
            with nc.allow_low_precision(reason="1/d broadcast in bf16 as before"):
                nc.vector.reciprocal(recd[:], dsum[:])
            fbs = att.tile([128, SEQ], bf16, tag="fbs", name=f"fbs{h}")
            nc.gpsimd.partition_broadcast(fbs[:], recd[0:1, :])
            nc.vector.tensor_mul(OT[h][:], OT[h][:], fbs[:])

        def load_slab(h, j):
            ks = att.tile([128, SLAB], bf16, tag="ks", name=f"ks{h}{j}")
            vsl = att.tile([128, VT, HD], bf16, tag="vsl", name=f"vsl{h}{j}")
            # split halves: the slab's first tiles land well before first use
            nc.sync.dma_start(ks[:, 0:1024],
                              ap["kTold"][h, :, j * SLAB:j * SLAB + 1024])
            nc.sync.dma_start(ks[:, 1024:SLAB],
                              ap["kTold"][h, :, j * SLAB + 1024:(j + 1) * SLAB])
            nc.sync.dma_start(vsl[:, 0:6, :], ap["vold"][h, j, :, 0:6, :])
            nc.sync.dma_start(vsl[:, 6:VT, :], ap["vold"][h, j, :, 6:VT, :])
            return (ks, vsl)

        # per-head sweep: pairs within slab (17 tiles -> 8 pairs + 1 single),
        # then new-token tiles -> 3 pairs; software pipeline depth 2 pairs;
        # slab j+1's DMA issued at the start of slab j; `fillers` emitted at
        # the given pair indices.
        PIPE = 12

        def sweep(h, slab0, fillers):
            state.update({"h": h, "first": True, "acc_first": True,
                          "o_ps": psO.tile([128, 1024], f32, tag="o",
                                           name=f"o_ps{h}")})
            pending = []
            pi = 0
            ret = {"nxt": None}

            def run_pair(pair):
                nonlocal pi
                e_t = emit_S_exp(pair)
                pending.append((pair, e_t))
                for fn in fillers.get(pi, ()):
                    fn()
                pi += 1
                # drain the PV backlog early as the tile stream runs out so
                # the accs don't trail the last exp by PIPE pairs
                while len(pending) > min(PIPE, max(1, len(fifo))):
                    pp, pe = pending.pop(0)
                    emit_PV_acc(pp, pe, False)

            # flat tile stream: cross-slab pairs avoid per-slab odd singles;
            # slab j+1's DMA issues as slab j's tiles enter the stream
            fifo = []
            loader = {"j": 0, "cur": slab0}

            def advance():
                j, (ks, vsl) = loader["j"], loader["cur"]
                if j + 1 < NSLAB:
                    loader["cur"] = load_slab(h, j + 1)
                elif h == 0:
                    loader["cur"] = load_slab(1, 0)
                    ret["nxt"] = loader["cur"]
                loader["j"] = j + 1
                for t in range(VT):
                    m = min(128, SLAB - t * 128)
                    fifo.append((ks[:, t * 128:t * 128 + m], vsl[0:m, t, :], m))
                if loader["j"] == NSLAB:
                    for st, (s0, m) in enumerate(S_TILES):
                        fifo.append((rk[h][:, s0:s0 + m],
                                     vs[st][0:m, h * HD:(h + 1) * HD], m))

            advance()
            while fifo:
                if len(fifo) <= VT and loader["j"] < NSLAB:
                    advance()
                if len(fifo) >= 2:
                    run_pair((fifo.pop(0), fifo.pop(0)))
                else:
                    run_pair((fifo.pop(0),))
            while pending:
                pp, pe = pending.pop(0)
                emit_PV_acc(pp, pe, not pending)
            return ret["nxt"]  # head 1's slab 0 when h == 0

        # o-projection, transposed: out^T[od, s] = sum_h OW_h[hd, od]^T @ OT_h.
        # 16 od-blocks of 128; per block 2 heads x 2 bank chunks accumulate in
        # a psS slot, DVE-evict to a rotating stage tile, DMA per block.
        out_r = ap["out"].rearrange("(n p) s -> p n s", p=128)
        ostate = {"stage": None}

        def oproj_block(b):
            if b % 3 == 2:   # tail-only: borrow the freed psO slot as 3rd buf
                op_ps = psO.tile([128, SEQ], f32, tag="o", name=f"opb{b}")
            else:
                op_ps = psS.tile([128, PW], f32, tag="pair", name=f"opb{b}")
            for h in range(HPC):
                for off, n in CHUNKS_720:
                    nc.tensor.matmul(op_ps[:, off:off + n],
                                     owt[:, h, b * 128:(b + 1) * 128],
                                     OT[h][:, off:off + n],
                                     start=(h == 0), stop=(h == HPC - 1))
            # evictions alternate DVE/ACT; stores go out two blocks per DMA
            # (halves the serialized per-store DGE overhead)
            if b % 2 == 0:
                ostate["stage"] = osb.tile([128, 2, SEQ], bf16, tag="ostage",
                                           name=f"ostage{b}")
                nc.vector.tensor_copy(ostate["stage"][:, 0, :], op_ps[:, 0:SEQ])
            else:
                nc.scalar.copy(ostate["stage"][:, 1, :], op_ps[:, 0:SEQ])
                nc.sync.dma_start(out_r[:, b - 1:b + 1, :], ostate["stage"][:])

        norm_factors("q", psO)
        final_mul("q")
        launch_collective("k")
        rope("k", 0)
        rope("k", 1)
        # wv lands after the latency-critical q-norm hops; first needed by
        # the v fillers a little into sweep 0
        nc.sync.dma_start(wv[:, 0:8, :], w_rs["wvT"][:, 0:8, :])
        nc.sync.dma_start(wv[:, 8:16, :], w_rs["wvT"][:, 8:16, :])
        norm_factors("k", psS)
        final_mul("k")
        if DEBUG:
            nc.sync.dma_start(ap["dbg_ln"][:], ln_t["q"][:])
            nc.sync.dma_start(ap["dbg_rq"][:], rq[0][:])
            nc.sync.dma_start(ap["dbg_rf"][:], recipf["q"][:])

        # head-0 fillers: head-0 v-tiles (pairs 0..5, before the first PV
        # zeroes the o-slot's banks) and k-RoPE/k-norm spread through the
        # sweep (all only needed by the new-token pairs at the sweep end;
        # the k-norm collective lands mid-sweep).
        f0 = defaultdict(list)
        for st in range(len(S_TILES)):
            f0[2 * st].append(lambda st=st: v_tile_h(st, 0))
        f0[11].append(late_loads)
        h1_slab0 = sweep(0, (ks0, vsl0), f0)

        # head-1 fillers: evict head-0's unnormalized O^T first (frees the
        # psO slot), then head-1 v-tiles in the pre-PV window; the rest of
        # head-0's denominator chain is PSUM-free and runs mid-sweep.
        o_ps0 = state["o_ps"]
        f1 = defaultdict(list)
        f1[0].append(lambda: evict_OTu(0, o_ps0))
        for st in range(len(S_TILES)):
            f1[1 + 2 * st].append(lambda st=st: v_tile_h(st, 1))
        f1[13].append(lambda: den_chain(0))
        sweep(1, h1_slab0, f1)

        # ---- tail: head-1 denominators (PE-based; PSUM is free now) then
        # the o-projection blocks + streamed stores ----
        evict_OTu(1, state["o_ps"])
        d_ps = psS.tile([128, PW], f32, tag="pair", name="d_ps1")
        for off, n in CHUNKS_720:
            nc.tensor.matmul(d_ps[0:1, off:off + n], ones_col[:],
                             accs[1][:, off:off + n], start=True, stop=False)
            nc.tensor.matmul(d_ps[0:1, off:off + n], ones_col[:],
                             accs[1][:, 720 + off:720 + off + n],
                             start=False, stop=True)
        recd1 = att.tile([1, SEQ], bf16, tag="rec_d", name="rec_d1")
        with nc.allow_low_precision(reason="1/d broadcast in bf16 as before"):
            nc.vector.reciprocal(recd1[:], d_ps[0:1, 0:SEQ])
        fb1 = psS.tile([128, PW], f32, tag="pair", name="fb1")
        fbs1 = att.tile([128, SEQ], bf16, tag="fbs", name="fbs1")
        for off, n in CHUNKS_720:
            nc.tensor.matmul(fb1[:, off:off + n], ones_row[:],
                             recd1[0:1, off:off + n], start=True, stop=True)
        nc.vector.tensor_copy(fbs1[:], fb1[:, 0:SEQ])
        nc.vector.tensor_mul(OT[1][:], OT[1][:], fbs1[:])
        for b in range(DIM // 128):
            oproj_block(b)
        if DEBUG:
            nc.sync.dma_start(ap["dbg_acc"][:], acc[:])
            nc.sync.dma_start(ap["dbg_ot0"][:], OT[0][:])
            nc.sync.dma_start(ap["dbg_ot1"][:], OT[1][:])
            nc.sync.dma_start(ap["dbg_rk"][:], rk[0][:])
            nc.sync.dma_start(ap["dbg_vs"][:], vs[0][:])


def _patch_act_tables(nc):
    """All ACT funcs used here (Exp, Ln, Copy) live in act-func-set 6
    (natural_log_exp_and_others); the auto-inserted per-function set loads
    thrash between exp/ln sets at ~1.3us per switch. Retarget every load to
    set 6 and drop redundant ones."""
    for blk in nc.main_func.blocks:
        keep = []
        seen = False
        for ins in blk.instructions:
            if isinstance(ins, mybir.InstLoadActFuncSet):
                ins.act_func_set_id = 6
                si = ins.sync_info
                clean = si is None or (len(si.on_wait) == 0 and len(si.on_update) == 0)
                if seen and clean:
                    continue  # redundant reload of the same set
                seen = True
            keep.append(ins)
        blk.instructions[:] = keep


def _dram_tensors(nc):
    d = {}
    d["xT"] = nc.dram_tensor("xT", [DIM, SEQ], bf16, kind="ExternalInput")
    d["wqT"] = nc.dram_tensor("wqT", [DIM, HDC], bf16, kind="ExternalInput")
    d["wkT"] = nc.dram_tensor("wkT", [DIM, HDC], bf16, kind="ExternalInput")
    d["wvT"] = nc.dram_tensor("wvT", [DIM, HDC], bf16, kind="ExternalInput")
    d["owT"] = nc.dram_tensor("owT", [HDC, DIM], bf16, kind="ExternalInput")
    d["qk_bias"] = nc.dram_tensor("qk_bias", [128, 4], f32, kind="ExternalInput")
    d["v_bias"] = nc.dram_tensor("v_bias", [1, HDC], bf16, kind="ExternalInput")
    d["ones_vr"] = nc.dram_tensor("ones_vr", [1, SEQ], bf16, kind="ExternalInput")
    d["cosq"] = nc.dram_tensor("cosq", [128, HPC * SEQ], bf16, kind="ExternalInput")
    d["sinq"] = nc.dram_tensor("sinq", [128, HPC * SEQ], bf16, kind="ExternalInput")
    d["cosk"] = nc.dram_tensor("cosk", [128, HPC * SEQ], bf16, kind="ExternalInput")
    d["sink"] = nc.dram_tensor("sink", [128, HPC * SEQ], bf16, kind="ExternalInput")
    d["kTold"] = nc.dram_tensor("kTold", [HPC, 128, OLD], bf16, kind="ExternalInput")
    d["vold"] = nc.dram_tensor("vold", [HPC, NSLAB, 128, VT, HD], bf16,
                               kind="ExternalInput")
    d["out"] = nc.dram_tensor("out", [DIM, SEQ], bf16, kind="ExternalOutput")
    if DEBUG:
        d["dbg_rq"] = nc.dram_tensor("dbg_rq", [128, SEQ], bf16, kind="ExternalOutput")
        d["dbg_rf"] = nc.dram_tensor("dbg_rf", [1, SEQ], bf16, kind="ExternalOutput")
        d["dbg_ln"] = nc.dram_tensor("dbg_ln", [1, SEQ], f32, kind="ExternalOutput")
        d["dbg_acc"] = nc.dram_tensor("dbg_acc", [128, 1440], bf16, kind="ExternalOutput")
        d["dbg_ot0"] = nc.dram_tensor("dbg_ot0", [128, SEQ], bf16, kind="ExternalOutput")
        d["dbg_ot1"] = nc.dram_tensor("dbg_ot1", [128, SEQ], bf16, kind="ExternalOutput")
        d["dbg_rk"] = nc.dram_tensor("dbg_rk", [128, SEQ], bf16, kind="ExternalOutput")
        d["dbg_vs"] = nc.dram_tensor("dbg_vs", [128, HDC], bf16, kind="ExternalOutput")
    return d


def _build():
    nc = bacc.Bacc("TRN2", target_bir_lowering=False, debug=False,
                   num_devices=N_CORES)
    d = _dram_tensors(nc)
    with tile.TileContext(nc) as tc:
        _emit(nc, tc, d)
    nc.compile()
    _patch_act_tables(nc)
    return nc


_NC_CACHE = None


def _get_nc():
    global _NC_CACHE
    if _NC_CACHE is None:
        _NC_CACHE = _build()
    return _NC_CACHE


def _bf(a):
    import ml_dtypes
    return np.asarray(a, dtype=np.float32).astype(ml_dtypes.bfloat16)


f8np_check = None


def _prep_inputs(x, q_w, q_b, k_w, k_b, v_w, v_b, o_w, o_b, norm_q_w, norm_k_w,
                 cache_k, cache_v, freqs_cos, freqs_sin,
                 current_start, frame_seqlen, sink_tokens):
    import ml_dtypes
    cs, sink = int(current_start), int(sink_tokens)
    rolling = CACHE - sink
    local_start = (cs - sink) % rolling + sink
    old_idx = np.r_[0:local_start, local_start + SEQ:CACHE]
    assert old_idx.size == OLD

    xT = _bf(np.ascontiguousarray(np.asarray(x)[0].T))

    # RoPE/norm tables in T layout: cos_full[d, s] = cos[s, d//2] * w[d];
    # sin_full[d, s] = sin[s, d//2] * w[d^1] * (-1 if d even else +1)
    dd = np.arange(HD)
    fc = np.asarray(freqs_cos, dtype=np.float32)
    fs = np.asarray(freqs_sin, dtype=np.float32)
    cos_d = fc.T[dd // 2, :]            # [128, 720]
    sin_d = fs.T[dd // 2, :]
    sign = np.where(dd % 2 == 0, -1.0, 1.0).astype(np.float32)[:, None]
    swap_m = np.zeros((HD, HD), dtype=np.float32)
    swap_m[dd, dd ^ 1] = 1.0

    ck = np.asarray(cache_k)[0]                # [11520, 16, 128]
    cv = np.asarray(cache_v)[0]
    ck_old = ck[old_idx]                       # [10800, 16, 128]
    cv_old = cv[old_idx]

    q_w, k_w, v_w, o_w = (np.asarray(a, dtype=np.float32)
                          for a in (q_w, k_w, v_w, o_w))
    q_b, k_b, v_b = (np.asarray(a, dtype=np.float32) for a in (q_b, k_b, v_b))

    in_maps = []
    for c in range(N_CORES):
        hs = slice(c * HDC, (c + 1) * HDC)
        heads = [c * HPC + h for h in range(HPC)]
        bias4 = np.zeros((128, 4), dtype=np.float32)
        for h in range(HPC):
            bias4[:, 0 + h] = q_b[hs][h * HD:(h + 1) * HD]
            bias4[:, 2 + h] = k_b[hs][h * HD:(h + 1) * HD]
        cosq = np.empty((128, HPC * SEQ), dtype=np.float32)
        sinq = np.empty((128, HPC * SEQ), dtype=np.float32)
        cosk = np.empty((128, HPC * SEQ), dtype=np.float32)
        sink_t = np.empty((128, HPC * SEQ), dtype=np.float32)
        for h in range(HPC):
            wqn = np.asarray(norm_q_w)[hs][h * HD:(h + 1) * HD]
            wkn = np.asarray(norm_k_w)[hs][h * HD:(h + 1) * HD]
            sl = slice(h * SEQ, (h + 1) * SEQ)
            cosq[:, sl] = cos_d * wqn[:, None]
            sinq[:, sl] = sin_d * wqn[dd ^ 1][:, None] * sign
            cosk[:, sl] = cos_d * wkn[:, None]
            sink_t[:, sl] = sin_d * wkn[dd ^ 1][:, None] * sign
        kT_old = np.ascontiguousarray(
            ck_old[:, heads, :].transpose(1, 2, 0))          # [2, 128, 10800]
        # vold packed to mirror the SBUF slab layout [h, j, p, t, e]
        vp = np.zeros((HPC, NSLAB, 128, VT, HD), dtype=ml_dtypes.bfloat16)
        for hi, head in enumerate(heads):
            v3 = cv_old[:, head, :].reshape(NSLAB, SLAB, HD)
            full = v3[:, :2048, :].reshape(NSLAB, 16, 128, HD)
            vp[hi, :, :, :16, :] = _bf(full.transpose(0, 2, 1, 3))
            vp[hi, :, :112, 16, :] = _bf(v3[:, 2048:, :])
        in_maps.append({
            "xT": xT,
            "wqT": _bf(q_w[hs, :].T),
            "wkT": _bf(k_w[hs, :].T),
            "wvT": _bf(v_w[hs, :].T),
            "owT": _bf(o_w[:, hs].T),
            "qk_bias": bias4,
            "v_bias": _bf(v_b[hs]).reshape(1, HDC),
            "ones_vr": np.ones((1, SEQ), dtype=ml_dtypes.bfloat16),
            "cosq": _bf(cosq), "sinq": _bf(sinq),
            "cosk": _bf(cosk), "sink": _bf(sink_t),
            "kTold": _bf(kT_old),
            "vold": vp,
        })
    return in_maps


def run_spmd(in_maps, **kw):
    nc = _get_nc()
    return bass_utils.run_bass_kernel_spmd(
        nc, in_maps, core_ids=list(range(N_CORES)), **kw)


def kernel(**inputs):
    inputs = {k: np.asarray(v) if not np.isscalar(v) else v
              for k, v in inputs.items()}
    in_maps = _prep_inputs(**inputs)
    res = run_spmd(in_maps)
    out = np.zeros((SEQ, DIM), dtype=np.float32)
    for c in range(N_CORES):
        out += np.asarray(res.results[c]["out"], dtype=np.float32).T
    out += np.asarray(inputs["o_b"], dtype=np.float32)[None, :]
    return out[None].astype(np.float32)



# revision 65
# speedup vs baseline: 1.0148x; 1.0148x over previous
"""CausalWanS2V self-attention — 8-core head-sharded Trainium2 Bass kernel.

Layout strategy (per core c, heads 2c..2c+1), v2 (bf16 + paired-exp):
  - All matmul operands bf16 (PSUM accumulation stays f32), so every matmul
    runs at 1 cycle/row regardless of chunk width and DMA bytes are halved.
  - q/k/v projections head-dim-major as before: qT/kT [hd=128, s=720] from
    host-transposed W^T / x^T tiles.
  - qk RMSNorm over the full 2048-dim vector via AllGather of per-core
    square-sum rows [1, 720] (bf16), summed with a Pool partition_all_reduce,
    rsqrt = exp(-0.5*ln(ms)) on ACT (stays in the natural_log_exp table set),
    then a Pool partition_broadcast feeds the per-column 1/rms factors to DVE.
  - Attention in S^T layout with PAIRED k-tiles: two 128-row chunks of cache
    positions share one PSUM tile [128, 1440] and one exp instruction,
    amortizing the ACT per-instruction overhead (the sweep is exp-throughput
    bound).  S chunks split at PSUM bank boundaries: tile A (0:512, 512:720),
    tile B (720:1024, 1024:1440).
  - Denominators: E pairs accumulate into a bf16 acc [128, 1440] on DVE
    (2-byte 2x mode); final column sums via Pool partition_all_reduce, then
    reciprocal + Pool partition_broadcast, all off the PE/ACT critical path.
  - Projections and RoPE all run before the first sweep; the k-norm collective
    round-trip and rk final multiplies are emitted as guarded fillers inside
    head 0's sweep (their deadline is the new-token tiles at the sweep end).
  - o-projection per-core partial [720, 2048]; host sums the 8 partials + o_b.
"""
import sys

sys.path.insert(0, "/opt/trn_rl_repo")

from collections import defaultdict

import numpy as np
import concourse.bass as bass
import concourse.bass_isa as bass_isa
import concourse.mybir as mybir
import concourse.tile as tile
from concourse import bacc
from concourse import bass_utils

f32 = mybir.dt.float32
bf16 = mybir.dt.bfloat16
f8 = mybir.dt.float8e4
DR = mybir.MatmulPerfMode.DoubleRowSwInterleave
AF = mybir.ActivationFunctionType
RADD = bass_isa.ReduceOp.add

# problem constants (hardcoded per contract)
SEQ = 720
DIM = 2048
NH = 16
HD = 128
CACHE = 11520
N_CORES = 8
HPC = NH // N_CORES        # heads per core = 2
HDC = HPC * HD             # 256 out dims per core
OLD = CACHE - SEQ          # 10800 old cache rows
SLAB = 2160                # kpos per DMA slab (5 slabs of 16*128+112)
NSLAB = OLD // SLAB
VT = (SLAB + 127) // 128   # 17 v-tiles per slab
SM_SCALE = float(HD) ** -0.5
EPS = 1e-6
KT = DIM // 128            # 16 contraction tiles
GP = KT // 2               # 8 DoubleRow contraction pairs for the q/k proj
W8S = 64.0                 # host pre-scale of fp8 q/k weights (rmsnorm-invariant)
PW = 1536                  # PSUM pair tile width (3 banks); cols 0:1440 used

DEBUG = False

# s-tiles of 720: 5 full 128s + one 80
S_TILES = [(i * 128, min(128, SEQ - i * 128)) for i in range((SEQ + 127) // 128)]
# in-bank chunking of a 720-col range starting at psum col 0 / col 720
CHUNKS_A = ((0, 512), (512, 208))
CHUNKS_B = ((720, 304), (1024, 416))
# chunking for [*, 720] psum tiles (projections etc.)
CHUNKS_720 = ((0, 512), (512, 208))


def _emit(nc, tc, d):
    """Emit the per-core program. d = dict of dram tensor handles."""
    ap = {k: v.ap() for k, v in d.items()}

    with tc.tile_pool(name="p0", bufs=1) as p0, \
         tc.tile_pool(name="dram", bufs=1, space="DRAM") as dpool, \
         tc.tile_pool(name="pa", bufs=1) as pa, \
         tc.tile_pool(name="att", bufs=2) as att, \
         tc.tile_pool(name="epool", bufs=13) as epool, \
         tc.tile_pool(name="osb", bufs=3) as osb, \
         tc.tile_pool(name="psS", bufs=2, space="PSUM") as psS, \
         tc.tile_pool(name="psO", bufs=1, space="PSUM") as psO:

        # ---- persistent tiles ----
        rq = [p0.tile([128, SEQ], bf16, tag=f"rq{h}", name=f"rq{h}") for h in range(HPC)]
        rk = [p0.tile([128, SEQ], bf16, tag=f"rk{h}", name=f"rk{h}") for h in range(HPC)]
        vs = [p0.tile([128, HDC], bf16, tag=f"vs{st}", name=f"vs{st}") for st in range(len(S_TILES))]
        OT = [p0.tile([128, SEQ], bf16, tag=f"ot{h}", name=f"ot{h}") for h in range(HPC)]
        accs = [p0.tile([128, 1440], bf16, tag=f"acc{h}", name=f"acc{h}")
                for h in range(HPC)]
        ones_col = p0.tile([128, 1], bf16, tag="ones_col")
        ones_row = p0.tile([1, 128], bf16, tag="ones_row")
        ones_row_f = p0.tile([1, 128], f32, tag="ones_row_f")
        one_one = p0.tile([1, 1], f32, tag="one_one")
        rec_col = [p0.tile([128, 8], f32, tag=f"rec{h}", name=f"rec{h}")
                   for h in range(HPC)]
        eps_t = p0.tile([1, 1], f32, tag="eps_t")
        prewarm = p0.tile([1, 1], f32, tag="prewarm")
        qb = {(tn, h): p0.tile([128, SEQ], bf16, tag=f"qb{tn}{h}", name=f"qb{tn}{h}")
              for tn in ("q", "k") for h in range(HPC)}
        gth = {tn: p0.tile([N_CORES, SEQ], bf16, tag=f"gth{tn}", name=f"gth{tn}")
               for tn in ("q", "k")}
        gsum = {tn: p0.tile([N_CORES, SEQ], f32, tag=f"gsum{tn}",
                            name=f"gsum{tn}") for tn in ("q", "k")}
        ln_t = {tn: p0.tile([1, SEQ], f32, tag=f"ln{tn}", name=f"ln{tn}")
                for tn in ("q", "k")}
        fbt = {tn: p0.tile([128, SEQ], bf16, tag=f"fbt{tn}", name=f"fbt{tn}")
               for tn in ("q", "k")}
        recipf = {tn: p0.tile([1, SEQ], bf16, tag=f"rf{tn}", name=f"rf{tn}")
                  for tn in ("q", "k")}
        owt = p0.tile([128, HPC, DIM], bf16, tag="owt")

        nc.gpsimd.memset(eps_t[:], EPS)
        nc.gpsimd.memset(ones_col[:], 1.0)
        nc.gpsimd.memset(ones_row[:], 1.0)
        nc.gpsimd.memset(ones_row_f[:], 1.0)
        nc.gpsimd.memset(one_one[:], 1.0)
        # pre-load the natural_log_exp table set while DMAs stream
        nc.scalar.activation(prewarm[:], eps_t[:], AF.Exp)

        # ---- phase A loads ----
        wq = pa.tile([128, KT, HDC], bf16, tag="wq")
        wk = pa.tile([128, KT, HDC], bf16, tag="wk")
        wv = pa.tile([128, KT, HDC], bf16, tag="wv")
        xt = pa.tile([128, KT, SEQ], bf16, tag="xt")
        cw = {nm: pa.tile([128, HPC * SEQ], bf16, tag=nm, name=nm)
              for nm in ("cosq", "sinq", "cosk", "sink")}
        bias_t = pa.tile([128, 4], f32, tag="bias")
        vb_t = pa.tile([1, HDC], bf16, tag="vb")
        ones_vr = pa.tile([1, SEQ], bf16, tag="ones_vr")

        x_r = ap["xT"].rearrange("(g p) s -> p g s", p=128)
        w_rs = {n: ap[n].rearrange("(g p) n -> p g n", p=128)
                for n in ("wqT", "wkT", "wvT")}
        # DMA issue order == SP-FIFO service order (single HWDGE device in
        # the cost model): q-proj inputs fine-grained, then k weights, first
        # half of slab-0 k (the sweep-start gate), q-rope tables, rest of
        # slab 0, v weights, k-rope tables.  owt is issued mid-sweep-0.
        for g in range(0, KT, 2):
            nc.sync.dma_start(wq[:, g:g + 2, :], w_rs["wqT"][:, g:g + 2, :])
            nc.sync.dma_start(xt[:, g:g + 2, :], x_r[:, g:g + 2, :])
        nc.sync.dma_start(bias_t[:], ap["qk_bias"])
        nc.sync.dma_start(cw["cosq"][:], ap["cosq"])
        nc.sync.dma_start(cw["sinq"][:], ap["sinq"])
        nc.sync.dma_start(wk[:, 0:8, :], w_rs["wkT"][:, 0:8, :])
        nc.sync.dma_start(wk[:, 8:16, :], w_rs["wkT"][:, 8:16, :])
        # slab 0 of head 0 preloaded into the att rotation (sweep-start gate);
        # the first half is the gate, the rest rides the ACT queue later.
        ks0 = att.tile([128, SLAB], bf16, tag="ks", name="ks00")
        vsl0 = att.tile([128, VT, HD], bf16, tag="vsl", name="vsl00")
        nc.sync.dma_start(ks0[:, 0:1024], ap["kTold"][0, :, 0:1024])
        nc.sync.dma_start(vb_t[:], ap["v_bias"])
        nc.sync.dma_start(ones_vr[:], ap["ones_vr"])
        nc.sync.dma_start(cw["cosk"][:], ap["cosk"])
        nc.sync.dma_start(cw["sink"][:], ap["sink"])

        def late_loads():
            nc.sync.dma_start(owt[:], ap["owT"].rearrange("(h p) n -> p h n", p=128))

        # ---- q then k projections, square-sums, collectives launched ASAP ----
        def proj_qk(tn, wt, ti):
            # both heads' psums allocated upfront so neither waits on the
            # other's eviction; the row reduction reuses head 0's slot
            pss = [psS.tile([128, PW], f32, tag="pair", name=f"ps_{tn}{h}")
                   for h in range(HPC)]
            for g in range(KT):
                for h in range(HPC):
                    for off, n in CHUNKS_720:
                        nc.tensor.matmul(
                            pss[h][:, off:off + n],
                            wt[:, g, h * HD:(h + 1) * HD],
                            xt[:, g, off:off + n],
                            start=(g == 0), stop=(g == KT - 1))
            sqs = []
            for h in range(HPC):
                nc.vector.tensor_scalar_add(qb[(tn, h)][:], pss[h][:, 0:SEQ],
                                            bias_t[:, 2 * ti + h:2 * ti + h + 1])
                sq = pa.tile([128, SEQ], bf16, tag=f"sq{h}", name=f"sq{tn}{h}")
                sqs.append(sq)
                nc.vector.tensor_mul(sq[:], qb[(tn, h)][:], qb[(tn, h)][:])
            row_ps = psO.tile([128, SEQ], f32, tag="o", name=f"row_{tn}")
            for h in range(HPC):
                for off, n in CHUNKS_720:
                    nc.tensor.matmul(row_ps[0:1, off:off + n], ones_col[:],
                                     sqs[h][:, off:off + n],
                                     start=(h == 0), stop=(h == HPC - 1))
            partial_sb = pa.tile([1, SEQ], bf16, tag=f"partial{tn}",
                                 name=f"partial{tn}")
            nc.vector.tensor_copy(partial_sb[0:1, :], row_ps[0:1, 0:SEQ])
            partials[tn] = partial_sb

        partials = {}

        def launch_collective(tn):
            bounce_in = dpool.tile([1, SEQ], bf16, name=f"bin{tn}")
            bounce_out = dpool.tile([N_CORES, SEQ], bf16, name=f"bout{tn}")
            # ACT-queue DMAs bypass the loaded SP FIFO (pre-sweep ACT is
            # idle); the k-side return must NOT ride ACT (it would
            # head-block the sweep exps until the collective lands).
            (nc.scalar if tn == "q" else nc.sync).dma_start(
                bounce_in[:], partials[tn][:])
            nc.gpsimd.collective_compute(
                "AllGather", mybir.AluOpType.bypass,
                replica_groups=[list(range(N_CORES))],
                ins=[bounce_in.opt()], outs=[bounce_out.opt()])
            if tn == "q":
                nc.scalar.dma_start(gth[tn][:], bounce_out[:])
            else:
                nc.sync.dma_start(gth[tn][:], bounce_out[:])

        proj_qk("q", wq, 0)

        # ---- RoPE: qb <- qb*cosW + swap(qb)*sinW; the pairwise partition
        # swap runs as two stride-2 sbuf->sbuf DMAs (no PE, no PSUM), so the
        # k-rope can run as a sweep filler.
        qbsw_t = {(tn, h): pa.tile([128, SEQ], bf16, tag=f"qbsw{tn}{h}",
                                   name=f"qbsw{tn}{h}")
                  for tn in ("q", "k") for h in range(HPC)}

        def rope_swap(tn, h):
            # q pre-sweep on the idle ACT queue (bypasses the SP FIFO); k is
            # not latency-critical and must keep off ACT (exp head-blocking).
            eng = nc.scalar if tn == "q" else nc.sync
            src, dst = qb[(tn, h)], qbsw_t[(tn, h)]
            eng.dma_start(dst[0:127:2, :], src[1:128:2, :])
            eng.dma_start(dst[1:128:2, :], src[0:127:2, :])

        def rope_muls(tn, h):
            cos_t = cw["cosq" if tn == "q" else "cosk"]
            sin_t = cw["sinq" if tn == "q" else "sink"]
            qbsw = qbsw_t[(tn, h)]
            t1 = pa.tile([128, SEQ], bf16, tag=f"t1{tn}{h}", name=f"t1{tn}{h}")
            nc.vector.tensor_mul(t1[:], qb[(tn, h)][:],
                                 cos_t[:, h * SEQ:(h + 1) * SEQ])
            nc.vector.tensor_mul(qbsw[:], qbsw[:],
                                 sin_t[:, h * SEQ:(h + 1) * SEQ])
            nc.vector.tensor_add(qb[(tn, h)][:], t1[:], qbsw[:])

        def rope(tn, h):
            rope_swap(tn, h)
            rope_muls(tn, h)

        # ---- norm factors: rsqrt(mean sq + eps) via Pool reduce + ACT ----
        def norm_factors(tn, pool=None):
            nc.gpsimd.partition_all_reduce(gsum[tn][:], gth[tn][:],
                                           channels=N_CORES, reduce_op=RADD)
            nc.scalar.activation(ln_t[tn][:], gsum[tn][0:1, :], AF.Ln,
                                 scale=1.0 / DIM, bias=eps_t[:])
            nc.scalar.activation(recipf[tn][:], ln_t[tn][:], AF.Exp,
                                 scale=-0.5)
            nc.gpsimd.partition_broadcast(fbt[tn][:], recipf[tn][0:1, :])

        def final_mul(tn):
            out_t = rq if tn == "q" else rk
            for h in range(HPC):
                nc.vector.tensor_mul(out_t[h][:], qb[(tn, h)][:], fbt[tn][:])

        # rope(q) with ACT evictions (ACT idle pre-sweep), then the norm-q
        # chain and rq finalization; the k-projection, v-projection and
        # k-RoPE all fill the q-collective round trip.
        rope("q", 0)
        rope("q", 1)
        launch_collective("q")
        # bulk slab-0 tail rides the ACT queue behind the q-norm bounce
        nc.scalar.dma_start(ks0[:, 1024:SLAB], ap["kTold"][0, :, 1024:SLAB])
        nc.scalar.dma_start(vsl0[:], ap["vold"][0, 0])
        proj_qk("k", wk, 1)

        # ---- v projection: one head x one s-tile per filler; accumulates in
        # the spare bank-1 region (cols 768:896) of the live o_ps tile, which
        # is only safe before the sweep's first PV (whose start=True zeroes
        # banks 0-1 of the slot) -- fillers go in the first pairs.
        def v_tile_h(st, h):
            s0, m = S_TILES[st]
            vp = state["o_ps"]
            for g in range(KT):
                nc.tensor.matmul(vp[0:m, 768:896], xt[:, g, s0:s0 + m],
                                 wv[:, g, h * HD:(h + 1) * HD],
                                 start=(g == 0), stop=False)
            nc.tensor.matmul(vp[0:m, 768:896], ones_vr[0:1, s0:s0 + m],
                             vb_t[:, h * HD:(h + 1) * HD], start=False, stop=True)
            nc.vector.tensor_copy(vs[st][0:m, h * HD:(h + 1) * HD],
                                  vp[0:m, 768:896])

        # ================= attention sweeps =================
        state = {"o_ps": None, "first": True}

        def emit_S_exp(pair):
            (kA, vA, mA) = pair[0]
            s_ps = psS.tile([128, PW], f32, tag="pair")
            for off, n in CHUNKS_A:
                nc.tensor.matmul(s_ps[0:mA, off:off + n], kA,
                                 rq[state["h"]][:, off:off + n],
                                 start=True, stop=True)
            if len(pair) > 1:
                (kB, vB, mB) = pair[1]
                for off, n in CHUNKS_B:
                    nc.tensor.matmul(s_ps[0:mB, off:off + n], kB,
                                     rq[state["h"]][:, off - 720:off - 720 + n],
                                     start=True, stop=True)
            else:
                mB = None
            e_t = epool.tile([128, PW], bf16, tag="e")
            if mB is None:
                nc.scalar.activation(e_t[0:mA, 0:SEQ], s_ps[0:mA, 0:SEQ],
                                     AF.Exp, scale=SM_SCALE)
            else:
                mm = max(mA, mB)
                nc.scalar.activation(e_t[0:mm, 0:1440], s_ps[0:mm, 0:1440],
                                     AF.Exp, scale=SM_SCALE)
            return e_t

        def emit_PV_acc(pair, e_t, last):
            o_ps = state["o_ps"]
            (kA, vA, mA) = pair[0]
            mB = pair[1][2] if len(pair) > 1 else None
            for off, n in CHUNKS_720:
                nc.tensor.matmul(o_ps[:, off:off + n], vA,
                                 e_t[0:mA, off:off + n],
                                 start=state["first"], stop=(last and mB is None))
            if mB is not None:
                vB = pair[1][1]
                for off, n in CHUNKS_720:
                    nc.tensor.matmul(o_ps[:, off:off + n], vB,
                                     e_t[0:mB, 720 + off:720 + off + n],
                                     start=False, stop=last)
            state["first"] = False
            # denominator accumulation on DVE (bf16 2x)
            acc = accs[state["h"]]
            eng = nc.vector
            if state["acc_first"]:
                state["acc_first"] = False
                if mB is not None and mA == mB:
                    eng.tensor_copy(acc[0:mA, :], e_t[0:mA, 0:1440])
                else:
                    eng.tensor_copy(acc[0:mA, 0:SEQ], e_t[0:mA, 0:SEQ])
                    if mB is not None:
                        eng.tensor_copy(acc[0:mB, 720:1440], e_t[0:mB, 720:1440])
            else:
                if mB is not None and mA == mB:
                    eng.tensor_add(acc[0:mA, :], acc[0:mA, :], e_t[0:mA, 0:1440])
                else:
                    eng.tensor_add(acc[0:mA, 0:SEQ], acc[0:mA, 0:SEQ],
                                   e_t[0:mA, 0:SEQ])
                    if mB is not None:
                        eng.tensor_add(acc[0:mB, 720:1440], acc[0:mB, 720:1440],
                                       e_t[0:mB, 720:1440])

        # ---- denominators, no PSUM: evict the unnormalized O^T first (frees
        # the psO slot), then reduce acc on the idle Pool engine, reciprocal
        # + partition-broadcast, and scale OT in place on DVE.
        dred = p0.tile([128, 1440], f32, tag="dred")
        dsum = p0.tile([1, SEQ], f32, tag="dsum")

        def evict_OTu(h, o_ps):
            nc.vector.tensor_copy(OT[h][:], o_ps[:, 0:SEQ])

        def den_chain(h):
            nc.gpsimd.partition_all_reduce(dred[:], accs[h][:],
                                           channels=128, reduce_op=RADD)
            nc.vector.tensor_add(dsum[:], dred[0:1, 0:SEQ],
                                 dred[0:1, 720:1440])
            recd = att.tile([1, SEQ], bf16, tag="rec_d", name=f"rec_d{h}")
            with nc.allow_low_precision(reason="1/d broadcast in bf16 as before"):
                nc.vector.reciprocal(recd[:], dsum[:])
            fbs = att.tile([128, SEQ], bf16, tag="fbs", name=f"fbs{h}")
            nc.gpsimd.partition_broadcast(fbs[:], recd[0:1, :])
            nc.vector.tensor_mul(OT[h][:], OT[h][:], fbs[:])

        def load_slab(h, j):
            ks = att.tile([128, SLAB], bf16, tag="ks", name=f"ks{h}{j}")
            vsl = att.tile([128, VT, HD], bf16, tag="vsl", name=f"vsl{h}{j}")
            # split halves: the slab's first tiles land well before first use
            nc.sync.dma_start(ks[:, 0:1024],
                              ap["kTold"][h, :, j * SLAB:j * SLAB + 1024])
            nc.sync.dma_start(ks[:, 1024:SLAB],
                              ap["kTold"][h, :, j * SLAB + 1024:(j + 1) * SLAB])
            nc.sync.dma_start(vsl[:, 0:6, :], ap["vold"][h, j, :, 0:6, :])
            nc.sync.dma_start(vsl[:, 6:VT, :], ap["vold"][h, j, :, 6:VT, :])
            return (ks, vsl)

        # per-head sweep: pairs within slab (17 tiles -> 8 pairs + 1 single),
        # then new-token tiles -> 3 pairs; software pipeline depth 2 pairs;
        # slab j+1's DMA issued at the start of slab j; `fillers` emitted at
        # the given pair indices.
        PIPE = 12

        def sweep(h, slab0, fillers):
            state.update({"h": h, "first": True, "acc_first": True,
                          "o_ps": psO.tile([128, 1024], f32, tag="o",
                                           name=f"o_ps{h}")})
            pending = []
            pi = 0
            ret = {"nxt": None}

            def run_pair(pair):
                nonlocal pi
                e_t = emit_S_exp(pair)
                pending.append((pair, e_t))
                for fn in fillers.get(pi, ()):
                    fn()
                pi += 1
                # drain the PV backlog early as the tile stream runs out so
                # the accs don't trail the last exp by PIPE pairs
                while len(pending) > min(PIPE, max(1, len(fifo))):
                    pp, pe = pending.pop(0)
                    emit_PV_acc(pp, pe, False)

            # flat tile stream: cross-slab pairs avoid per-slab odd singles;
            # slab j+1's DMA issues as slab j's tiles enter the stream
            fifo = []
            loader = {"j": 0, "cur": slab0}

            def advance():
                j, (ks, vsl) = loader["j"], loader["cur"]
                if j + 1 < NSLAB:
                    loader["cur"] = load_slab(h, j + 1)
                elif h == 0:
                    loader["cur"] = load_slab(1, 0)
                    ret["nxt"] = loader["cur"]
                loader["j"] = j + 1
                for t in range(VT):
                    m = min(128, SLAB - t * 128)
                    fifo.append((ks[:, t * 128:t * 128 + m], vsl[0:m, t, :], m))
                if loader["j"] == NSLAB:
                    for st, (s0, m) in enumerate(S_TILES):
                        fifo.append((rk[h][:, s0:s0 + m],
                                     vs[st][0:m, h * HD:(h + 1) * HD], m))

            advance()
            while fifo:
                if len(fifo) <= VT and loader["j"] < NSLAB:
                    advance()
                if len(fifo) >= 2:
                    run_pair((fifo.pop(0), fifo.pop(0)))
                else:
                    run_pair((fifo.pop(0),))
            while pending:
                pp, pe = pending.pop(0)
                emit_PV_acc(pp, pe, not pending)
            return ret["nxt"]  # head 1's slab 0 when h == 0

        # o-projection, transposed: out^T[od, s] = sum_h OW_h[hd, od]^T @ OT_h.
        # 16 od-blocks of 128; per block 2 heads x 2 bank chunks accumulate in
        # a psS slot, DVE-evict to a rotating stage tile, DMA per block.
        out_r = ap["out"].rearrange("(n p) s -> p n s", p=128)
        ostate = {"stage": None}

        def oproj_block(b):
            if b % 3 == 2:   # tail-only: borrow the freed psO slot as 3rd buf
                op_ps = psO.tile([128, SEQ], f32, tag="o", name=f"opb{b}")
            else:
                op_ps = psS.tile([128, PW], f32, tag="pair", name=f"opb{b}")
            for h in range(HPC):
                for off, n in CHUNKS_720:
                    nc.tensor.matmul(op_ps[:, off:off + n],
                                     owt[:, h, b * 128:(b + 1) * 128],
                                     OT[h][:, off:off + n],
                                     start=(h == 0), stop=(h == HPC - 1))
            # evictions alternate DVE/ACT; stores go out two blocks per DMA
            # (halves the serialized per-store DGE overhead)
            if b % 2 == 0:
                ostate["stage"] = osb.tile([128, 2, SEQ], bf16, tag="ostage",
                                           name=f"ostage{b}")
                nc.vector.tensor_copy(ostate["stage"][:, 0, :], op_ps[:, 0:SEQ])
            else:
                nc.scalar.copy(ostate["stage"][:, 1, :], op_ps[:, 0:SEQ])
                nc.sync.dma_start(out_r[:, b - 1:b + 1, :], ostate["stage"][:])

        norm_factors("q", psO)
        final_mul("q")
        launch_collective("k")
        rope("k", 0)
        rope("k", 1)
        # wv lands after the latency-critical q-norm hops; first needed by
        # the v fillers a little into sweep 0
        nc.sync.dma_start(wv[:, 0:8, :], w_rs["wvT"][:, 0:8, :])
        nc.sync.dma_start(wv[:, 8:16, :], w_rs["wvT"][:, 8:16, :])
        norm_factors("k", psS)
        final_mul("k")
        if DEBUG:
            nc.sync.dma_start(ap["dbg_ln"][:], ln_t["q"][:])
            nc.sync.dma_start(ap["dbg_rq"][:], rq[0][:])
            nc.sync.dma_start(ap["dbg_rf"][:], recipf["q"][:])

        # head-0 fillers: head-0 v-tiles (pairs 0..5, before the first PV
        # zeroes the o-slot's banks) and k-RoPE/k-norm spread through the
        # sweep (all only needed by the new-token pairs at the sweep end;
        # the k-norm collective lands mid-sweep).
        f0 = defaultdict(list)
        for st in range(len(S_TILES)):
            f0[2 * st].append(lambda st=st: v_tile_h(st, 0))
        f0[11].append(late_loads)
        h1_slab0 = sweep(0, (ks0, vsl0), f0)

        # head-1 fillers: evict head-0's unnormalized O^T first (frees the
        # psO slot), then head-1 v-tiles in the pre-PV window; the rest of
        # head-0's denominator chain is PSUM-free and runs mid-sweep.
        o_ps0 = state["o_ps"]
        f1 = defaultdict(list)
        f1[0].append(lambda: evict_OTu(0, o_ps0))
        for st in range(len(S_TILES)):
            f1[1 + 2 * st].append(lambda st=st: v_tile_h(st, 1))
        f1[13].append(lambda: den_chain(0))
        sweep(1, h1_slab0, f1)

        # ---- tail: head-1 denominators (PE-based; PSUM is free now) then
        # the o-projection blocks + streamed stores ----
        evict_OTu(1, state["o_ps"])
        d_ps = psS.tile([128, PW], f32, tag="pair", name="d_ps1")
        for off, n in CHUNKS_720:
            nc.tensor.matmul(d_ps[0:1, off:off + n], ones_col[:],
                             accs[1][:, off:off + n], start=True, stop=False)
            nc.tensor.matmul(d_ps[0:1, off:off + n], ones_col[:],
                             accs[1][:, 720 + off:720 + off + n],
                             start=False, stop=True)
        recd1 = att.tile([1, SEQ], bf16, tag="rec_d", name="rec_d1")
        with nc.allow_low_precision(reason="1/d broadcast in bf16 as before"):
            nc.vector.reciprocal(recd1[:], d_ps[0:1, 0:SEQ])
        fb1 = psS.tile([128, PW], f32, tag="pair", name="fb1")
        fbs1 = att.tile([128, SEQ], bf16, tag="fbs", name="fbs1")
        for off, n in CHUNKS_720:
            nc.tensor.matmul(fb1[:, off:off + n], ones_row[:],
                             recd1[0:1, off:off + n], start=True, stop=True)
        nc.vector.tensor_copy(fbs1[:], fb1[:, 0:SEQ])
        nc.vector.tensor_mul(OT[1][:], OT[1][:], fbs1[:])
        for b in range(DIM // 128):
            oproj_block(b)
        if DEBUG:
            nc.sync.dma_start(ap["dbg_acc"][:], acc[:])
            nc.sync.dma_start(ap["dbg_ot0"][:], OT[0][:])
            nc.sync.dma_start(ap["dbg_ot1"][:], OT[1][:])
            nc.sync.dma_start(ap["dbg_rk"][:], rk[0][:])
            nc.sync.dma_start(ap["dbg_vs"][:], vs[0][:])


def _patch_act_tables(nc):
    """All ACT funcs used here (Exp, Ln, Copy) live in act-func-set 6
    (natural_log_exp_and_others); the auto-inserted per-function set loads
    thrash between exp/ln sets at ~1.3us per switch. Retarget every load to
    set 6 and drop redundant ones."""
    for blk in nc.main_func.blocks:
        keep = []
        seen = False
        for ins in blk.instructions:
            if isinstance(ins, mybir.InstLoadActFuncSet):
                ins.act_func_set_id = 6
                si = ins.sync_info
                clean = si is None or (len(si.on_wait) == 0 and len(si.on_update) == 0)
                if seen and clean:
                    continue  # redundant reload of the same set
                seen = True
            keep.append(ins)
        blk.instructions[:] = keep


def _dram_tensors(nc):
    d = {}
    d["xT"] = nc.dram_tensor("xT", [DIM, SEQ], bf16, kind="ExternalInput")
    d["wqT"] = nc.dram_tensor("wqT", [DIM, HDC], bf16, kind="ExternalInput")
    d["wkT"] = nc.dram_tensor("wkT", [DIM, HDC], bf16, kind="ExternalInput")
    d["wvT"] = nc.dram_tensor("wvT", [DIM, HDC], bf16, kind="ExternalInput")
    d["owT"] = nc.dram_tensor("owT", [HDC, DIM], bf16, kind="ExternalInput")
    d["qk_bias"] = nc.dram_tensor("qk_bias", [128, 4], f32, kind="ExternalInput")
    d["v_bias"] = nc.dram_tensor("v_bias", [1, HDC], bf16, kind="ExternalInput")
    d["ones_vr"] = nc.dram_tensor("ones_vr", [1, SEQ], bf16, kind="ExternalInput")
    d["cosq"] = nc.dram_tensor("cosq", [128, HPC * SEQ], bf16, kind="ExternalInput")
    d["sinq"] = nc.dram_tensor("sinq", [128, HPC * SEQ], bf16, kind="ExternalInput")
    d["cosk"] = nc.dram_tensor("cosk", [128, HPC * SEQ], bf16, kind="ExternalInput")
    d["sink"] = nc.dram_tensor("sink", [128, HPC * SEQ], bf16, kind="ExternalInput")
    d["kTold"] = nc.dram_tensor("kTold", [HPC, 128, OLD], bf16, kind="ExternalInput")
    d["vold"] = nc.dram_tensor("vold", [HPC, NSLAB, 128, VT, HD], bf16,
                               kind="ExternalInput")
    d["out"] = nc.dram_tensor("out", [DIM, SEQ], bf16, kind="ExternalOutput")
    if DEBUG:
        d["dbg_rq"] = nc.dram_tensor("dbg_rq", [128, SEQ], bf16, kind="ExternalOutput")
        d["dbg_rf"] = nc.dram_tensor("dbg_rf", [1, SEQ], bf16, kind="ExternalOutput")
        d["dbg_ln"] = nc.dram_tensor("dbg_ln", [1, SEQ], f32, kind="ExternalOutput")
        d["dbg_acc"] = nc.dram_tensor("dbg_acc", [128, 1440], bf16, kind="ExternalOutput")
        d["dbg_ot0"] = nc.dram_tensor("dbg_ot0", [128, SEQ], bf16, kind="ExternalOutput")
        d["dbg_ot1"] = nc.dram_tensor("dbg_ot1", [128, SEQ], bf16, kind="ExternalOutput")
        d["dbg_rk"] = nc.dram_tensor("dbg_rk", [128, SEQ], bf16, kind="ExternalOutput")
        d["dbg_vs"] = nc.dram_tensor("dbg_vs", [128, HDC], bf16, kind="ExternalOutput")
    return d


def _build():
    nc = bacc.Bacc("TRN2", target_bir_lowering=False, debug=False,
                   num_devices=N_CORES)
    d = _dram_tensors(nc)
    with tile.TileContext(nc) as tc:
        _emit(nc, tc, d)
    nc.compile()
    _patch_act_tables(nc)
    return nc


_NC_CACHE = None


def _get_nc():
    global _NC_CACHE
    if _NC_CACHE is None:
        _NC_CACHE = _build()
    return _NC_CACHE


def _bf(a):
    import ml_dtypes
    return np.asarray(a, dtype=np.float32).astype(ml_dtypes.bfloat16)


f8np_check = None


def _prep_inputs(x, q_w, q_b, k_w, k_b, v_w, v_b, o_w, o_b, norm_q_w, norm_k_w,
                 cache_k, cache_v, freqs_cos, freqs_sin,
                 current_start, frame_seqlen, sink_tokens):
    import ml_dtypes
    cs, sink = int(current_start), int(sink_tokens)
    rolling = CACHE - sink
    local_start = (cs - sink) % rolling + sink
    old_idx = np.r_[0:local_start, local_start + SEQ:CACHE]
    assert old_idx.size == OLD

    xT = _bf(np.ascontiguousarray(np.asarray(x)[0].T))

    # RoPE/norm tables in T layout: cos_full[d, s] = cos[s, d//2] * w[d];
    # sin_full[d, s] = sin[s, d//2] * w[d^1] * (-1 if d even else +1)
    dd = np.arange(HD)
    fc = np.asarray(freqs_cos, dtype=np.float32)
    fs = np.asarray(freqs_sin, dtype=np.float32)
    cos_d = fc.T[dd // 2, :]            # [128, 720]
    sin_d = fs.T[dd // 2, :]
    sign = np.where(dd % 2 == 0, -1.0, 1.0).astype(np.float32)[:, None]
    swap_m = np.zeros((HD, HD), dtype=np.float32)
    swap_m[dd, dd ^ 1] = 1.0

    ck = np.asarray(cache_k)[0]                # [11520, 16, 128]
    cv = np.asarray(cache_v)[0]
    ck_old = ck[old_idx]                       # [10800, 16, 128]
    cv_old = cv[old_idx]

    q_w, k_w, v_w, o_w = (np.asarray(a, dtype=np.float32)
                          for a in (q_w, k_w, v_w, o_w))
    q_b, k_b, v_b = (np.asarray(a, dtype=np.float32) for a in (q_b, k_b, v_b))

    in_maps = []
    for c in range(N_CORES):
        hs = slice(c * HDC, (c + 1) * HDC)
        heads = [c * HPC + h for h in range(HPC)]
        bias4 = np.zeros((128, 4), dtype=np.float32)
        for h in range(HPC):
            bias4[:, 0 + h] = q_b[hs][h * HD:(h + 1) * HD]
            bias4[:, 2 + h] = k_b[hs][h * HD:(h + 1) * HD]
        cosq = np.empty((128, HPC * SEQ), dtype=np.float32)
        sinq = np.empty((128, HPC * SEQ), dtype=np.float32)
        cosk = np.empty((128, HPC * SEQ), dtype=np.float32)
        sink_t = np.empty((128, HPC * SEQ), dtype=np.float32)
        for h in range(HPC):
            wqn = np.asarray(norm_q_w)[hs][h * HD:(h + 1) * HD]
            wkn = np.asarray(norm_k_w)[hs][h * HD:(h + 1) * HD]
            sl = slice(h * SEQ, (h + 1) * SEQ)
            cosq[:, sl] = cos_d * wqn[:, None]
            sinq[:, sl] = sin_d * wqn[dd ^ 1][:, None] * sign
            cosk[:, sl] = cos_d * wkn[:, None]
            sink_t[:, sl] = sin_d * wkn[dd ^ 1][:, None] * sign
        kT_old = np.ascontiguousarray(
            ck_old[:, heads, :].transpose(1, 2, 0))          # [2, 128, 10800]
        # vold packed to mirror the SBUF slab layout [h, j, p, t, e]
        vp = np.zeros((HPC, NSLAB, 128, VT, HD), dtype=ml_dtypes.bfloat16)
        for hi, head in enumerate(heads):
            v3 = cv_old[:, head, :].reshape(NSLAB, SLAB, HD)
            full = v3[:, :2048, :].reshape(NSLAB, 16, 128, HD)
            vp[hi, :, :, :16, :] = _bf(full.transpose(0, 2, 1, 3))
            vp[hi, :, :112, 16, :] = _bf(v3[:, 2048:, :])
        in_maps.append({
            "xT": xT,
            "wqT": _bf(q_w[hs, :].T),
            "wkT": _bf(k_w[hs, :].T),
            "wvT": _bf(v_w[hs, :].T),
            "owT": _bf(o_w[:, hs].T),
            "qk_bias": bias4,
            "v_bias": _bf(v_b[hs]).reshape(1, HDC),
            "ones_vr": np.ones((1, SEQ), dtype=ml_dtypes.bfloat16),
            "cosq": _bf(cosq), "sinq": _bf(sinq),
            "cosk": _bf(cosk), "sink": _bf(sink_t),
            "kTold": _bf(kT_old),
            "vold": vp,
        })
    return in_maps


def run_spmd(in_maps, **kw):
    nc = _get_nc()
    return bass_utils.run_bass_kernel_spmd(
        nc, in_maps, core_ids=list(range(N_CORES)), **kw)


def kernel(**inputs):
    inputs = {k: np.asarray(v) if not np.isscalar(v) else v
              for k, v in inputs.items()}
    in_maps = _prep_inputs(**inputs)
    res = run_spmd(in_maps)
    out = np.zeros((SEQ, DIM), dtype=np.float32)
    for c in range(N_CORES):
        out += np.asarray(res.results[c]["out"], dtype=np.float32).T
    out += np.asarray(inputs["o_b"], dtype=np.float32)[None, :]
    return out[None].astype(np.float32)



# revision 67
# speedup vs baseline: 1.0477x; 1.0324x over previous
"""CausalWanS2V self-attention — 8-core head-sharded Trainium2 Bass kernel.

Layout strategy (per core c, heads 2c..2c+1), v2 (bf16 + paired-exp):
  - All matmul operands bf16 (PSUM accumulation stays f32), so every matmul
    runs at 1 cycle/row regardless of chunk width and DMA bytes are halved.
  - q/k/v projections head-dim-major as before: qT/kT [hd=128, s=720] from
    host-transposed W^T / x^T tiles.
  - qk RMSNorm over the full 2048-dim vector via AllGather of per-core
    square-sum rows [1, 720] (bf16), summed with a Pool partition_all_reduce,
    rsqrt = exp(-0.5*ln(ms)) on ACT (stays in the natural_log_exp table set),
    then a Pool partition_broadcast feeds the per-column 1/rms factors to DVE.
  - Attention in S^T layout with PAIRED k-tiles: two 128-row chunks of cache
    positions share one PSUM tile [128, 1440] and one exp instruction,
    amortizing the ACT per-instruction overhead (the sweep is exp-throughput
    bound).  S chunks split at PSUM bank boundaries: tile A (0:512, 512:720),
    tile B (720:1024, 1024:1440).
  - Denominators: E pairs accumulate into a bf16 acc [128, 1440] on DVE
    (2-byte 2x mode); final column sums via Pool partition_all_reduce, then
    reciprocal + Pool partition_broadcast, all off the PE/ACT critical path.
  - Projections and RoPE all run before the first sweep; the k-norm collective
    round-trip and rk final multiplies are emitted as guarded fillers inside
    head 0's sweep (their deadline is the new-token tiles at the sweep end).
  - o-projection per-core partial [720, 2048]; host sums the 8 partials + o_b.
"""
import sys

sys.path.insert(0, "/opt/trn_rl_repo")

from collections import defaultdict

import numpy as np
import concourse.bass as bass
import concourse.bass_isa as bass_isa
import concourse.mybir as mybir
import concourse.tile as tile
from concourse import bacc
from concourse import bass_utils

f32 = mybir.dt.float32
bf16 = mybir.dt.bfloat16
f8 = mybir.dt.float8e4
DR = mybir.MatmulPerfMode.DoubleRowSwInterleave
AF = mybir.ActivationFunctionType
RADD = bass_isa.ReduceOp.add

# problem constants (hardcoded per contract)
SEQ = 720
DIM = 2048
NH = 16
HD = 128
CACHE = 11520
N_CORES = 8
HPC = NH // N_CORES        # heads per core = 2
HDC = HPC * HD             # 256 out dims per core
OLD = CACHE - SEQ          # 10800 old cache rows
SLAB = 2160                # kpos per DMA slab (5 slabs of 16*128+112)
NSLAB = OLD // SLAB
VT = (SLAB + 127) // 128   # 17 v-tiles per slab
SM_SCALE = float(HD) ** -0.5
EPS = 1e-6
KT = DIM // 128            # 16 contraction tiles
GP = KT // 2               # 8 DoubleRow contraction pairs for the q/k proj
W8S = 64.0                 # host pre-scale of fp8 q/k weights (rmsnorm-invariant)
PW = 1536                  # PSUM pair tile width (3 banks); cols 0:1440 used

DEBUG = False

# s-tiles of 720: 5 full 128s + one 80
S_TILES = [(i * 128, min(128, SEQ - i * 128)) for i in range((SEQ + 127) // 128)]
# in-bank chunking of a 720-col range starting at psum col 0 / col 720
CHUNKS_A = ((0, 512), (512, 208))
CHUNKS_B = ((720, 304), (1024, 416))
# chunking for [*, 720] psum tiles (projections etc.)
CHUNKS_720 = ((0, 512), (512, 208))


def _emit(nc, tc, d):
    """Emit the per-core program. d = dict of dram tensor handles."""
    ap = {k: v.ap() for k, v in d.items()}

    with tc.tile_pool(name="p0", bufs=1) as p0, \
         tc.tile_pool(name="dram", bufs=1, space="DRAM") as dpool, \
         tc.tile_pool(name="pa", bufs=1) as pa, \
         tc.tile_pool(name="att", bufs=2) as att, \
         tc.tile_pool(name="epool", bufs=13) as epool, \
         tc.tile_pool(name="osb", bufs=3) as osb, \
         tc.tile_pool(name="psS", bufs=2, space="PSUM") as psS, \
         tc.tile_pool(name="psO", bufs=1, space="PSUM") as psO:

        # ---- persistent tiles ----
        rq = [p0.tile([128, SEQ], bf16, tag=f"rq{h}", name=f"rq{h}") for h in range(HPC)]
        rk = [p0.tile([128, SEQ], bf16, tag=f"rk{h}", name=f"rk{h}") for h in range(HPC)]
        vs = [p0.tile([128, HDC], bf16, tag=f"vs{st}", name=f"vs{st}") for st in range(len(S_TILES))]
        OT = [p0.tile([128, SEQ], bf16, tag=f"ot{h}", name=f"ot{h}") for h in range(HPC)]
        accs = [p0.tile([128, 1440], bf16, tag=f"acc{h}", name=f"acc{h}")
                for h in range(HPC)]
        ones_col = p0.tile([128, 1], bf16, tag="ones_col")
        ones_row = p0.tile([1, 128], bf16, tag="ones_row")
        ones_row_f = p0.tile([1, 128], f32, tag="ones_row_f")
        one_one = p0.tile([1, 1], f32, tag="one_one")
        rec_col = [p0.tile([128, 8], f32, tag=f"rec{h}", name=f"rec{h}")
                   for h in range(HPC)]
        eps_t = p0.tile([1, 1], f32, tag="eps_t")
        prewarm = p0.tile([1, 1], f32, tag="prewarm")
        qb = {(tn, h): p0.tile([128, SEQ], bf16, tag=f"qb{tn}{h}", name=f"qb{tn}{h}")
              for tn in ("q", "k") for h in range(HPC)}
        gth = {tn: p0.tile([N_CORES, SEQ], bf16, tag=f"gth{tn}", name=f"gth{tn}")
               for tn in ("q", "k")}
        gsum = {tn: p0.tile([N_CORES, SEQ], f32, tag=f"gsum{tn}",
                            name=f"gsum{tn}") for tn in ("q", "k")}
        ln_t = {tn: p0.tile([1, SEQ], f32, tag=f"ln{tn}", name=f"ln{tn}")
                for tn in ("q", "k")}
        fbt = {tn: p0.tile([128, SEQ], bf16, tag=f"fbt{tn}", name=f"fbt{tn}")
               for tn in ("q", "k")}
        recipf = {tn: p0.tile([1, SEQ], bf16, tag=f"rf{tn}", name=f"rf{tn}")
                  for tn in ("q", "k")}
        owt = p0.tile([128, HPC, DIM], bf16, tag="owt")

        nc.gpsimd.memset(eps_t[:], EPS)
        nc.gpsimd.memset(ones_col[:], 1.0)
        nc.gpsimd.memset(ones_row[:], 1.0)
        nc.gpsimd.memset(ones_row_f[:], 1.0)
        nc.gpsimd.memset(one_one[:], 1.0)
        # pre-load the natural_log_exp table set while DMAs stream
        nc.scalar.activation(prewarm[:], eps_t[:], AF.Exp)

        # ---- phase A loads ----
        wq = pa.tile([128, KT, HDC], bf16, tag="wq")
        wk = pa.tile([128, KT, HDC], bf16, tag="wk")
        wv = pa.tile([128, KT, HDC], bf16, tag="wv")
        xt = pa.tile([128, KT, SEQ], bf16, tag="xt")
        cw = {nm: pa.tile([128, HPC * SEQ], bf16, tag=nm, name=nm)
              for nm in ("cosq", "sinq", "cosk", "sink")}
        bias_t = pa.tile([128, 4], f32, tag="bias")
        vb_t = pa.tile([1, HDC], bf16, tag="vb")
        ones_vr = pa.tile([1, SEQ], bf16, tag="ones_vr")

        x_r = ap["xT"].rearrange("(g p) s -> p g s", p=128)
        w_rs = {n: ap[n].rearrange("(g p) n -> p g n", p=128)
                for n in ("wqT", "wkT", "wvT")}
        # DMA issue order == SP-FIFO service order (single HWDGE device in
        # the cost model): q-proj inputs fine-grained, then k weights, first
        # half of slab-0 k (the sweep-start gate), q-rope tables, rest of
        # slab 0, v weights, k-rope tables.  owt is issued mid-sweep-0.
        for g in range(0, KT, 2):
            nc.sync.dma_start(wq[:, g:g + 2, :], w_rs["wqT"][:, g:g + 2, :])
            nc.sync.dma_start(xt[:, g:g + 2, :], x_r[:, g:g + 2, :])
        nc.sync.dma_start(bias_t[:], ap["qk_bias"])
        nc.sync.dma_start(cw["cosq"][:], ap["cosq"])
        nc.sync.dma_start(cw["sinq"][:], ap["sinq"])
        nc.sync.dma_start(wk[:, 0:8, :], w_rs["wkT"][:, 0:8, :])
        nc.sync.dma_start(wk[:, 8:16, :], w_rs["wkT"][:, 8:16, :])
        # slab 0 of head 0 preloaded into the att rotation (sweep-start gate);
        # the first half is the gate, the rest rides the ACT queue later.
        ks0 = att.tile([128, SLAB], bf16, tag="ks", name="ks00")
        vsl0 = att.tile([128, VT, HD], bf16, tag="vsl", name="vsl00")
        nc.sync.dma_start(ks0[:, 0:1024], ap["kTold"][0, :, 0:1024])
        nc.sync.dma_start(wv[:, 0:8, :], w_rs["wvT"][:, 0:8, :])
        nc.sync.dma_start(wv[:, 8:16, :], w_rs["wvT"][:, 8:16, :])
        nc.sync.dma_start(vb_t[:], ap["v_bias"])
        nc.sync.dma_start(ones_vr[:], ap["ones_vr"])
        nc.sync.dma_start(cw["cosk"][:], ap["cosk"])
        nc.sync.dma_start(cw["sink"][:], ap["sink"])

        def late_loads():
            nc.sync.dma_start(owt[:], ap["owT"].rearrange("(h p) n -> p h n", p=128))

        # ---- q then k projections, square-sums, collectives launched ASAP ----
        def proj_qk(tn, wt, ti):
            # both heads' psums allocated upfront so neither waits on the
            # other's eviction; the row reduction reuses head 0's slot
            pss = [psS.tile([128, PW], f32, tag="pair", name=f"ps_{tn}{h}")
                   for h in range(HPC)]
            for g in range(KT):
                for h in range(HPC):
                    for off, n in CHUNKS_720:
                        nc.tensor.matmul(
                            pss[h][:, off:off + n],
                            wt[:, g, h * HD:(h + 1) * HD],
                            xt[:, g, off:off + n],
                            start=(g == 0), stop=(g == KT - 1))
            sqs = []
            for h in range(HPC):
                nc.vector.tensor_scalar_add(qb[(tn, h)][:], pss[h][:, 0:SEQ],
                                            bias_t[:, 2 * ti + h:2 * ti + h + 1])
                sq = pa.tile([128, SEQ], bf16, tag=f"sq{h}", name=f"sq{tn}{h}")
                sqs.append(sq)
                nc.vector.tensor_mul(sq[:], qb[(tn, h)][:], qb[(tn, h)][:])
            row_ps = psO.tile([128, SEQ], f32, tag="o", name=f"row_{tn}")
            for h in range(HPC):
                for off, n in CHUNKS_720:
                    nc.tensor.matmul(row_ps[0:1, off:off + n], ones_col[:],
                                     sqs[h][:, off:off + n],
                                     start=(h == 0), stop=(h == HPC - 1))
            partial_sb = pa.tile([1, SEQ], bf16, tag=f"partial{tn}",
                                 name=f"partial{tn}")
            nc.vector.tensor_copy(partial_sb[0:1, :], row_ps[0:1, 0:SEQ])
            partials[tn] = partial_sb

        partials = {}

        def launch_collective(tn):
            bounce_in = dpool.tile([1, SEQ], bf16, name=f"bin{tn}")
            bounce_out = dpool.tile([N_CORES, SEQ], bf16, name=f"bout{tn}")
            # ACT-queue DMAs bypass the loaded SP FIFO (pre-sweep ACT is
            # idle); the k-side return must NOT ride ACT (it would
            # head-block the sweep exps until the collective lands).
            (nc.scalar if tn == "q" else nc.sync).dma_start(
                bounce_in[:], partials[tn][:])
            nc.gpsimd.collective_compute(
                "AllGather", mybir.AluOpType.bypass,
                replica_groups=[list(range(N_CORES))],
                ins=[bounce_in.opt()], outs=[bounce_out.opt()])
            if tn == "q":
                nc.scalar.dma_start(gth[tn][:], bounce_out[:])
            else:
                nc.sync.dma_start(gth[tn][:], bounce_out[:])

        proj_qk("q", wq, 0)

        # ---- RoPE: qb <- qb*cosW + swap(qb)*sinW; the pairwise partition
        # swap runs as two stride-2 sbuf->sbuf DMAs (no PE, no PSUM), so the
        # k-rope can run as a sweep filler.
        qbsw_t = {(tn, h): pa.tile([128, SEQ], bf16, tag=f"qbsw{tn}{h}",
                                   name=f"qbsw{tn}{h}")
                  for tn in ("q", "k") for h in range(HPC)}

        def rope_swap(tn, h):
            # q pre-sweep on the idle ACT queue (bypasses the SP FIFO); k is
            # not latency-critical and must keep off ACT (exp head-blocking).
            eng = nc.scalar if tn == "q" else nc.sync
            src, dst = qb[(tn, h)], qbsw_t[(tn, h)]
            eng.dma_start(dst[0:127:2, :], src[1:128:2, :])
            eng.dma_start(dst[1:128:2, :], src[0:127:2, :])

        def rope_muls(tn, h):
            cos_t = cw["cosq" if tn == "q" else "cosk"]
            sin_t = cw["sinq" if tn == "q" else "sink"]
            qbsw = qbsw_t[(tn, h)]
            t1 = pa.tile([128, SEQ], bf16, tag=f"t1{tn}{h}", name=f"t1{tn}{h}")
            nc.vector.tensor_mul(t1[:], qb[(tn, h)][:],
                                 cos_t[:, h * SEQ:(h + 1) * SEQ])
            nc.vector.tensor_mul(qbsw[:], qbsw[:],
                                 sin_t[:, h * SEQ:(h + 1) * SEQ])
            nc.vector.tensor_add(qb[(tn, h)][:], t1[:], qbsw[:])

        def rope(tn, h):
            rope_swap(tn, h)
            rope_muls(tn, h)

        # ---- norm factors: rsqrt(mean sq + eps) via Pool reduce + ACT ----
        def norm_factors(tn, pool=None):
            nc.gpsimd.partition_all_reduce(gsum[tn][:], gth[tn][:],
                                           channels=N_CORES, reduce_op=RADD)
            nc.scalar.activation(ln_t[tn][:], gsum[tn][0:1, :], AF.Ln,
                                 scale=1.0 / DIM, bias=eps_t[:])
            nc.scalar.activation(recipf[tn][:], ln_t[tn][:], AF.Exp,
                                 scale=-0.5)
            nc.gpsimd.partition_broadcast(fbt[tn][:], recipf[tn][0:1, :])

        def final_mul(tn):
            out_t = rq if tn == "q" else rk
            for h in range(HPC):
                nc.vector.tensor_mul(out_t[h][:], qb[(tn, h)][:], fbt[tn][:])

        # rope(q) with ACT evictions (ACT idle pre-sweep), then the norm-q
        # chain and rq finalization; the k-projection, v-projection and
        # k-RoPE all fill the q-collective round trip.
        rope("q", 0)
        rope("q", 1)
        launch_collective("q")
        # bulk slab-0 tail rides the ACT queue behind the q-norm bounce
        nc.scalar.dma_start(ks0[:, 1024:SLAB], ap["kTold"][0, :, 1024:SLAB])
        nc.scalar.dma_start(vsl0[:], ap["vold"][0, 0])
        proj_qk("k", wk, 1)

        # ---- v projection: one head x one s-tile per filler; accumulates in
        # the spare bank-1 region (cols 768:896) of the live o_ps tile, which
        # is only safe before the sweep's first PV (whose start=True zeroes
        # banks 0-1 of the slot) -- fillers go in the first pairs.
        def v_tile_h(st, h):
            s0, m = S_TILES[st]
            vp = state["o_ps"]
            for g in range(KT):
                nc.tensor.matmul(vp[0:m, 768:896], xt[:, g, s0:s0 + m],
                                 wv[:, g, h * HD:(h + 1) * HD],
                                 start=(g == 0), stop=False)
            nc.tensor.matmul(vp[0:m, 768:896], ones_vr[0:1, s0:s0 + m],
                             vb_t[:, h * HD:(h + 1) * HD], start=False, stop=True)
            nc.vector.tensor_copy(vs[st][0:m, h * HD:(h + 1) * HD],
                                  vp[0:m, 768:896])

        # ================= attention sweeps =================
        state = {"o_ps": None, "first": True}

        def emit_S_exp(pair):
            (kA, vA, mA) = pair[0]
            s_ps = psS.tile([128, PW], f32, tag="pair")
            for off, n in CHUNKS_A:
                nc.tensor.matmul(s_ps[0:mA, off:off + n], kA,
                                 rq[state["h"]][:, off:off + n],
                                 start=True, stop=True)
            if len(pair) > 1:
                (kB, vB, mB) = pair[1]
                for off, n in CHUNKS_B:
                    nc.tensor.matmul(s_ps[0:mB, off:off + n], kB,
                                     rq[state["h"]][:, off - 720:off - 720 + n],
                                     start=True, stop=True)
            else:
                mB = None
            e_t = epool.tile([128, PW], bf16, tag="e")
            if mB is None:
                nc.scalar.activation(e_t[0:mA, 0:SEQ], s_ps[0:mA, 0:SEQ],
                                     AF.Exp, scale=SM_SCALE)
            else:
                mm = max(mA, mB)
                nc.scalar.activation(e_t[0:mm, 0:1440], s_ps[0:mm, 0:1440],
                                     AF.Exp, scale=SM_SCALE)
            return e_t

        def emit_PV_acc(pair, e_t, last):
            o_ps = state["o_ps"]
            (kA, vA, mA) = pair[0]
            mB = pair[1][2] if len(pair) > 1 else None
            for off, n in CHUNKS_720:
                nc.tensor.matmul(o_ps[:, off:off + n], vA,
                                 e_t[0:mA, off:off + n],
                                 start=state["first"], stop=(last and mB is None))
            if mB is not None:
                vB = pair[1][1]
                for off, n in CHUNKS_720:
                    nc.tensor.matmul(o_ps[:, off:off + n], vB,
                                     e_t[0:mB, 720 + off:720 + off + n],
                                     start=False, stop=last)
            state["first"] = False
            # denominator accumulation on DVE (bf16 2x)
            acc = accs[state["h"]]
            eng = nc.vector
            if state["acc_first"]:
                state["acc_first"] = False
                if mB is not None and mA == mB:
                    eng.tensor_copy(acc[0:mA, :], e_t[0:mA, 0:1440])
                else:
                    eng.tensor_copy(acc[0:mA, 0:SEQ], e_t[0:mA, 0:SEQ])
                    if mB is not None:
                        eng.tensor_copy(acc[0:mB, 720:1440], e_t[0:mB, 720:1440])
            else:
                if mB is not None and mA == mB:
                    eng.tensor_add(acc[0:mA, :], acc[0:mA, :], e_t[0:mA, 0:1440])
                else:
                    eng.tensor_add(acc[0:mA, 0:SEQ], acc[0:mA, 0:SEQ],
                                   e_t[0:mA, 0:SEQ])
                    if mB is not None:
                        eng.tensor_add(acc[0:mB, 720:1440], acc[0:mB, 720:1440],
                                       e_t[0:mB, 720:1440])

        # ---- denominators, no PSUM: evict the unnormalized O^T first (frees
        # the psO slot), then reduce acc on the idle Pool engine, reciprocal
        # + partition-broadcast, and scale OT in place on DVE.
        dred = p0.tile([128, 1440], f32, tag="dred")
        dsum = p0.tile([1, SEQ], f32, tag="dsum")

        def evict_OTu(h, o_ps):
            nc.vector.tensor_copy(OT[h][:], o_ps[:, 0:SEQ])

        def den_chain(h):
            nc.gpsimd.partition_all_reduce(dred[:], accs[h][:],
                                           channels=128, reduce_op=RADD)
            nc.vector.tensor_add(dsum[:], dred[0:1, 0:SEQ],
                                 dred[0:1, 720:1440])
            recd = att.tile([1, SEQ], bf16, tag="rec_d", name=f"rec_d{h}")
            with nc.allow_low_precision(reason="1/d broadcast in bf16 as before"):
                nc.vector.reciprocal(recd[:], dsum[:])
            fbs = att.tile([128, SEQ], bf16, tag="fbs", name=f"fbs{h}")
            nc.gpsimd.partition_broadcast(fbs[:], recd[0:1, :])
            nc.vector.tensor_mul(OT[h][:], OT[h][:], fbs[:])

        def load_slab(h, j):
            ks = att.tile([128, SLAB], bf16, tag="ks", name=f"ks{h}{j}")
            vsl = att.tile([128, VT, HD], bf16, tag="vsl", name=f"vsl{h}{j}")
            # split halves: the slab's first tiles land well before first use
            nc.sync.dma_start(ks[:, 0:1024],
                              ap["kTold"][h, :, j * SLAB:j * SLAB + 1024])
            nc.sync.dma_start(ks[:, 1024:SLAB],
                              ap["kTold"][h, :, j * SLAB + 1024:(j + 1) * SLAB])
            nc.sync.dma_start(vsl[:, 0:6, :], ap["vold"][h, j, :, 0:6, :])
            nc.sync.dma_start(vsl[:, 6:VT, :], ap["vold"][h, j, :, 6:VT, :])
            return (ks, vsl)

        # per-head sweep: pairs within slab (17 tiles -> 8 pairs + 1 single),
        # then new-token tiles -> 3 pairs; software pipeline depth 2 pairs;
        # slab j+1's DMA issued at the start of slab j; `fillers` emitted at
        # the given pair indices.
        PIPE = 12

        def sweep(h, slab0, fillers):
            state.update({"h": h, "first": True, "acc_first": True,
                          "o_ps": psO.tile([128, 1024], f32, tag="o",
                                           name=f"o_ps{h}")})
            pending = []
            pi = 0
            ret = {"nxt": None}

            def run_pair(pair):
                nonlocal pi
                e_t = emit_S_exp(pair)
                pending.append((pair, e_t))
                for fn in fillers.get(pi, ()):
                    fn()
                pi += 1
                # drain the PV backlog early as the tile stream runs out so
                # the accs don't trail the last exp by PIPE pairs
                while len(pending) > min(PIPE, max(1, len(fifo))):
                    pp, pe = pending.pop(0)
                    emit_PV_acc(pp, pe, False)

            # flat tile stream: cross-slab pairs avoid per-slab odd singles;
            # slab j+1's DMA issues as slab j's tiles enter the stream
            fifo = []
            loader = {"j": 0, "cur": slab0}

            def advance():
                j, (ks, vsl) = loader["j"], loader["cur"]
                if j + 1 < NSLAB:
                    loader["cur"] = load_slab(h, j + 1)
                elif h == 0:
                    loader["cur"] = load_slab(1, 0)
                    ret["nxt"] = loader["cur"]
                loader["j"] = j + 1
                for t in range(VT):
                    m = min(128, SLAB - t * 128)
                    fifo.append((ks[:, t * 128:t * 128 + m], vsl[0:m, t, :], m))
                if loader["j"] == NSLAB:
                    for st, (s0, m) in enumerate(S_TILES):
                        fifo.append((rk[h][:, s0:s0 + m],
                                     vs[st][0:m, h * HD:(h + 1) * HD], m))

            advance()
            while fifo:
                if len(fifo) <= VT and loader["j"] < NSLAB:
                    advance()
                if len(fifo) >= 2:
                    run_pair((fifo.pop(0), fifo.pop(0)))
                else:
                    run_pair((fifo.pop(0),))
            while pending:
                pp, pe = pending.pop(0)
                emit_PV_acc(pp, pe, not pending)
            return ret["nxt"]  # head 1's slab 0 when h == 0

        # o-projection, transposed: out^T[od, s] = sum_h OW_h[hd, od]^T @ OT_h.
        # 16 od-blocks of 128; per block 2 heads x 2 bank chunks accumulate in
        # a psS slot, DVE-evict to a rotating stage tile, DMA per block.
        out_r = ap["out"].rearrange("(n p) s -> p n s", p=128)
        ostate = {"stage": None}

        def oproj_block(b):
            if b % 3 == 2:   # tail-only: borrow the freed psO slot as 3rd buf
                op_ps = psO.tile([128, SEQ], f32, tag="o", name=f"opb{b}")
            else:
                op_ps = psS.tile([128, PW], f32, tag="pair", name=f"opb{b}")
            for h in range(HPC):
                for off, n in CHUNKS_720:
                    nc.tensor.matmul(op_ps[:, off:off + n],
                                     owt[:, h, b * 128:(b + 1) * 128],
                                     OT[h][:, off:off + n],
                                     start=(h == 0), stop=(h == HPC - 1))
            # evictions alternate DVE/ACT; stores go out two blocks per DMA
            # (halves the serialized per-store DGE overhead)
            if b % 2 == 0:
                ostate["stage"] = osb.tile([128, 2, SEQ], bf16, tag="ostage",
                                           name=f"ostage{b}")
                nc.vector.tensor_copy(ostate["stage"][:, 0, :], op_ps[:, 0:SEQ])
            else:
                nc.scalar.copy(ostate["stage"][:, 1, :], op_ps[:, 0:SEQ])
                nc.sync.dma_start(out_r[:, b - 1:b + 1, :], ostate["stage"][:])

        norm_factors("q", psO)
        final_mul("q")
        launch_collective("k")
        rope("k", 0)
        rope("k", 1)
        norm_factors("k", psS)
        final_mul("k")
        if DEBUG:
            nc.sync.dma_start(ap["dbg_ln"][:], ln_t["q"][:])
            nc.sync.dma_start(ap["dbg_rq"][:], rq[0][:])
            nc.sync.dma_start(ap["dbg_rf"][:], recipf["q"][:])

        # head-0 fillers: head-0 v-tiles (pairs 0..5, before the first PV
        # zeroes the o-slot's banks) and k-RoPE/k-norm spread through the
        # sweep (all only needed by the new-token pairs at the sweep end;
        # the k-norm collective lands mid-sweep).
        f0 = defaultdict(list)
        for st in range(len(S_TILES)):
            f0[2 * st].append(lambda st=st: v_tile_h(st, 0))
        f0[11].append(late_loads)
        h1_slab0 = sweep(0, (ks0, vsl0), f0)

        # head-1 fillers: evict head-0's unnormalized O^T first (frees the
        # psO slot), then head-1 v-tiles in the pre-PV window; the rest of
        # head-0's denominator chain is PSUM-free and runs mid-sweep.
        o_ps0 = state["o_ps"]
        f1 = defaultdict(list)
        f1[0].append(lambda: evict_OTu(0, o_ps0))
        for st in range(len(S_TILES)):
            f1[1 + 2 * st].append(lambda st=st: v_tile_h(st, 1))
        f1[13].append(lambda: den_chain(0))
        sweep(1, h1_slab0, f1)

        # ---- tail: head-1 denominators (PE-based; PSUM is free now) then
        # the o-projection blocks + streamed stores ----
        evict_OTu(1, state["o_ps"])
        d_ps = psS.tile([128, PW], f32, tag="pair", name="d_ps1")
        for off, n in CHUNKS_720:
            nc.tensor.matmul(d_ps[0:1, off:off + n], ones_col[:],
                             accs[1][:, off:off + n], start=True, stop=False)
            nc.tensor.matmul(d_ps[0:1, off:off + n], ones_col[:],
                             accs[1][:, 720 + off:720 + off + n],
                             start=False, stop=True)
        recd1 = att.tile([1, SEQ], bf16, tag="rec_d", name="rec_d1")
        with nc.allow_low_precision(reason="1/d broadcast in bf16 as before"):
            nc.vector.reciprocal(recd1[:], d_ps[0:1, 0:SEQ])
        fb1 = psS.tile([128, PW], f32, tag="pair", name="fb1")
        fbs1 = att.tile([128, SEQ], bf16, tag="fbs", name="fbs1")
        for off, n in CHUNKS_720:
            nc.tensor.matmul(fb1[:, off:off + n], ones_row[:],
                             recd1[0:1, off:off + n], start=True, stop=True)
        nc.vector.tensor_copy(fbs1[:], fb1[:, 0:SEQ])
        nc.vector.tensor_mul(OT[1][:], OT[1][:], fbs1[:])
        for b in range(DIM // 128):
            oproj_block(b)
        if DEBUG:
            nc.sync.dma_start(ap["dbg_acc"][:], acc[:])
            nc.sync.dma_start(ap["dbg_ot0"][:], OT[0][:])
            nc.sync.dma_start(ap["dbg_ot1"][:], OT[1][:])
            nc.sync.dma_start(ap["dbg_rk"][:], rk[0][:])
            nc.sync.dma_start(ap["dbg_vs"][:], vs[0][:])


def _patch_act_tables(nc):
    """All ACT funcs used here (Exp, Ln, Copy) live in act-func-set 6
    (natural_log_exp_and_others); the auto-inserted per-function set loads
    thrash between exp/ln sets at ~1.3us per switch. Retarget every load to
    set 6 and drop redundant ones."""
    for blk in nc.main_func.blocks:
        keep = []
        seen = False
        for ins in blk.instructions:
            if isinstance(ins, mybir.InstLoadActFuncSet):
                ins.act_func_set_id = 6
                si = ins.sync_info
                clean = si is None or (len(si.on_wait) == 0 and len(si.on_update) == 0)
                if seen and clean:
                    continue  # redundant reload of the same set
                seen = True
            keep.append(ins)
        blk.instructions[:] = keep


def _dram_tensors(nc):
    d = {}
    d["xT"] = nc.dram_tensor("xT", [DIM, SEQ], bf16, kind="ExternalInput")
    d["wqT"] = nc.dram_tensor("wqT", [DIM, HDC], bf16, kind="ExternalInput")
    d["wkT"] = nc.dram_tensor("wkT", [DIM, HDC], bf16, kind="ExternalInput")
    d["wvT"] = nc.dram_tensor("wvT", [DIM, HDC], bf16, kind="ExternalInput")
    d["owT"] = nc.dram_tensor("owT", [HDC, DIM], bf16, kind="ExternalInput")
    d["qk_bias"] = nc.dram_tensor("qk_bias", [128, 4], f32, kind="ExternalInput")
    d["v_bias"] = nc.dram_tensor("v_bias", [1, HDC], bf16, kind="ExternalInput")
    d["ones_vr"] = nc.dram_tensor("ones_vr", [1, SEQ], bf16, kind="ExternalInput")
    d["cosq"] = nc.dram_tensor("cosq", [128, HPC * SEQ], bf16, kind="ExternalInput")
    d["sinq"] = nc.dram_tensor("sinq", [128, HPC * SEQ], bf16, kind="ExternalInput")
    d["cosk"] = nc.dram_tensor("cosk", [128, HPC * SEQ], bf16, kind="ExternalInput")
    d["sink"] = nc.dram_tensor("sink", [128, HPC * SEQ], bf16, kind="ExternalInput")
    d["kTold"] = nc.dram_tensor("kTold", [HPC, 128, OLD], bf16, kind="ExternalInput")
    d["vold"] = nc.dram_tensor("vold", [HPC, NSLAB, 128, VT, HD], bf16,
                               kind="ExternalInput")
    d["out"] = nc.dram_tensor("out", [DIM, SEQ], bf16, kind="ExternalOutput")
    if DEBUG:
        d["dbg_rq"] = nc.dram_tensor("dbg_rq", [128, SEQ], bf16, kind="ExternalOutput")
        d["dbg_rf"] = nc.dram_tensor("dbg_rf", [1, SEQ], bf16, kind="ExternalOutput")
        d["dbg_ln"] = nc.dram_tensor("dbg_ln", [1, SEQ], f32, kind="ExternalOutput")
        d["dbg_acc"] = nc.dram_tensor("dbg_acc", [128, 1440], bf16, kind="ExternalOutput")
        d["dbg_ot0"] = nc.dram_tensor("dbg_ot0", [128, SEQ], bf16, kind="ExternalOutput")
        d["dbg_ot1"] = nc.dram_tensor("dbg_ot1", [128, SEQ], bf16, kind="ExternalOutput")
        d["dbg_rk"] = nc.dram_tensor("dbg_rk", [128, SEQ], bf16, kind="ExternalOutput")
        d["dbg_vs"] = nc.dram_tensor("dbg_vs", [128, HDC], bf16, kind="ExternalOutput")
    return d


def _build():
    nc = bacc.Bacc("TRN2", target_bir_lowering=False, debug=False,
                   num_devices=N_CORES)
    d = _dram_tensors(nc)
    with tile.TileContext(nc) as tc:
        _emit(nc, tc, d)
    nc.compile()
    _patch_act_tables(nc)
    return nc


_NC_CACHE = None


def _get_nc():
    global _NC_CACHE
    if _NC_CACHE is None:
        _NC_CACHE = _build()
    return _NC_CACHE


def _bf(a):
    import ml_dtypes
    return np.asarray(a, dtype=np.float32).astype(ml_dtypes.bfloat16)


f8np_check = None


def _prep_inputs(x, q_w, q_b, k_w, k_b, v_w, v_b, o_w, o_b, norm_q_w, norm_k_w,
                 cache_k, cache_v, freqs_cos, freqs_sin,
                 current_start, frame_seqlen, sink_tokens):
    import ml_dtypes
    cs, sink = int(current_start), int(sink_tokens)
    rolling = CACHE - sink
    local_start = (cs - sink) % rolling + sink
    old_idx = np.r_[0:local_start, local_start + SEQ:CACHE]
    assert old_idx.size == OLD

    xT = _bf(np.ascontiguousarray(np.asarray(x)[0].T))

    # RoPE/norm tables in T layout: cos_full[d, s] = cos[s, d//2] * w[d];
    # sin_full[d, s] = sin[s, d//2] * w[d^1] * (-1 if d even else +1)
    dd = np.arange(HD)
    fc = np.asarray(freqs_cos, dtype=np.float32)
    fs = np.asarray(freqs_sin, dtype=np.float32)
    cos_d = fc.T[dd // 2, :]            # [128, 720]
    sin_d = fs.T[dd // 2, :]
    sign = np.where(dd % 2 == 0, -1.0, 1.0).astype(np.float32)[:, None]
    swap_m = np.zeros((HD, HD), dtype=np.float32)
    swap_m[dd, dd ^ 1] = 1.0

    ck = np.asarray(cache_k)[0]                # [11520, 16, 128]
    cv = np.asarray(cache_v)[0]
    ck_old = ck[old_idx]                       # [10800, 16, 128]
    cv_old = cv[old_idx]

    q_w, k_w, v_w, o_w = (np.asarray(a, dtype=np.float32)
                          for a in (q_w, k_w, v_w, o_w))
    q_b, k_b, v_b = (np.asarray(a, dtype=np.float32) for a in (q_b, k_b, v_b))

    in_maps = []
    for c in range(N_CORES):
        hs = slice(c * HDC, (c + 1) * HDC)
        heads = [c * HPC + h for h in range(HPC)]
        bias4 = np.zeros((128, 4), dtype=np.float32)
        for h in range(HPC):
            bias4[:, 0 + h] = q_b[hs][h * HD:(h + 1) * HD]
            bias4[:, 2 + h] = k_b[hs][h * HD:(h + 1) * HD]
        cosq = np.empty((128, HPC * SEQ), dtype=np.float32)
        sinq = np.empty((128, HPC * SEQ), dtype=np.float32)
        cosk = np.empty((128, HPC * SEQ), dtype=np.float32)
        sink_t = np.empty((128, HPC * SEQ), dtype=np.float32)
        for h in range(HPC):
            wqn = np.asarray(norm_q_w)[hs][h * HD:(h + 1) * HD]
            wkn = np.asarray(norm_k_w)[hs][h * HD:(h + 1) * HD]
            sl = slice(h * SEQ, (h + 1) * SEQ)
            cosq[:, sl] = cos_d * wqn[:, None]
            sinq[:, sl] = sin_d * wqn[dd ^ 1][:, None] * sign
            cosk[:, sl] = cos_d * wkn[:, None]
            sink_t[:, sl] = sin_d * wkn[dd ^ 1][:, None] * sign
        kT_old = np.ascontiguousarray(
            ck_old[:, heads, :].transpose(1, 2, 0))          # [2, 128, 10800]
        # vold packed to mirror the SBUF slab layout [h, j, p, t, e]
        vp = np.zeros((HPC, NSLAB, 128, VT, HD), dtype=ml_dtypes.bfloat16)
        for hi, head in enumerate(heads):
            v3 = cv_old[:, head, :].reshape(NSLAB, SLAB, HD)
            full = v3[:, :2048, :].reshape(NSLAB, 16, 128, HD)
            vp[hi, :, :, :16, :] = _bf(full.transpose(0, 2, 1, 3))
            vp[hi, :, :112, 16, :] = _bf(v3[:, 2048:, :])
        in_maps.append({
            "xT": xT,
            "wqT": _bf(q_w[hs, :].T),
            "wkT": _bf(k_w[hs, :].T),
            "wvT": _bf(v_w[hs, :].T),
            "owT": _bf(o_w[:, hs].T),
            "qk_bias": bias4,
            "v_bias": _bf(v_b[hs]).reshape(1, HDC),
            "ones_vr": np.ones((1, SEQ), dtype=ml_dtypes.bfloat16),
            "cosq": _bf(cosq), "sinq": _bf(sinq),
            "cosk": _bf(cosk), "sink": _bf(sink_t),
            "kTold": _bf(kT_old),
            "vold": vp,
        })
    return in_maps


def run_spmd(in_maps, **kw):
    nc = _get_nc()
    return bass_utils.run_bass_kernel_spmd(
        nc, in_maps, core_ids=list(range(N_CORES)), **kw)


def kernel(**inputs):
    inputs = {k: np.asarray(v) if not np.isscalar(v) else v
              for k, v in inputs.items()}
    in_maps = _prep_inputs(**inputs)
    res = run_spmd(in_maps)
    out = np.zeros((SEQ, DIM), dtype=np.float32)
    for c in range(N_CORES):
        out += np.asarray(res.results[c]["out"], dtype=np.float32).T
    out += np.asarray(inputs["o_b"], dtype=np.float32)[None, :]
    return out[None].astype(np.float32)

